# revision 17
# baseline (speedup 1.0000x reference)
"""Trainium2 Bass kernel for nn_CardGNN (3-layer GATv2 message passing), v3.

Sharding: nodes partitioned across 8 NeuronCores (6250 each, 50 blocks of 125
dst nodes). Each core computes xr = h @ Wr for its OWN nodes only and the
node-major bf16 table is AllGathered (replaces v2's 8x-redundant table
compute). Per-edge source features are fetched with dma_gather (bulk 256B
rows, int16 indices, table split at row 32750), round-robining 4 SWDGE
queues. Self-loop features come from an SBUF-resident copy (no gather, and
exact per-block-slot subtile counts instead of a global max). The x_i one-hot
expansion and the x_i+x_j add both run on the TensorEngine accumulating in
PSUM; leaky-relu reads PSUM on the Scalar engine. Segment softmax +
aggregation are one-hot matmuls accumulating in PSUM.
"""
import math
import numpy as np
import ml_dtypes

import concourse.bacc as bacc
import concourse.mybir as mybir
import concourse.tile as tile
from concourse.bass_utils import run_bass_kernel_spmd

F32 = mybir.dt.float32
BF16 = mybir.dt.bfloat16
I16 = mybir.dt.int16
AF = mybir.ActivationFunctionType
OP = mybir.AluOpType

N = 50000
E = 800000
IN = 128
HID = 32
HEADS = 4
CH = 32
HC = HID * HEADS  # 128
EMB = 128
NCORES = 8
NPC = N // NCORES          # 6250 nodes per core
BN = 125                   # dst nodes per block
NBLK = NPC // BN           # 50 blocks per core
NGB = N // BN              # 400 global blocks
P = 128
SPLIT = 32750              # lo/hi table split so idx fits int16
NEG = 0.2
LAYERS = 3
import os as _os
NQ = int(_os.environ.get("V3NQ", "4"))  # SWDGE queues
ETCH = 4                   # et psum chunk, subtiles

_PLAN = None


class Plan:
    pass


def _wrap16(idx_flat):
    """dma_gather index layout: w[p, s] = idx[s*16+p], replicated to 128 rows."""
    w = idx_flat.reshape(-1, 16).T.astype(np.int16)
    return np.tile(w, (8, 1))


def _preprocess(edge_index):
    global _PLAN
    src = np.asarray(edge_index[0]).astype(np.int64)
    dst = np.asarray(edge_index[1]).astype(np.int64)
    order = np.argsort(dst, kind="stable")
    src = src[order]
    dst = dst[order]
    blk = dst // BN
    starts = np.searchsorted(blk, np.arange(NGB))
    ends = np.searchsorted(blk, np.arange(NGB) + 1)

    lo, hi = [], []
    for g in range(NGB):
        s, e = starts[g], ends[g]
        sg, dg = src[s:e], dst[s:e]
        m = sg < SPLIT
        lo.append((sg[m], dg[m]))
        hi.append((sg[~m] - SPLIT, dg[~m]))

    gl = np.zeros(NBLK, np.int64)
    gh = np.zeros(NBLK, np.int64)
    for g in range(NGB):
        b = g % NBLK
        gl[b] = max(gl[b], math.ceil(len(lo[g][0]) / P))
        gh[b] = max(gh[b], math.ceil(len(hi[g][0]) / P))
    gsub = gl + gh
    nsub = gsub + 1          # + self subtile

    plan = Plan()
    plan.gl, plan.gh, plan.gsub, plan.nsub = gl, gh, gsub, nsub
    plan.off8 = np.concatenate([[0], np.cumsum(8 * gsub)])
    plan.offn = np.concatenate([[0], np.cumsum(nsub)])
    plan.TOTIDX8 = int(plan.off8[-1])
    plan.TOTNS = int(plan.offn[-1])
    plan.MAXNS = int(nsub.max())

    idxm = np.zeros((NCORES, P, plan.TOTIDX8), np.int16)
    dlm = np.full((NCORES, P, plan.TOTNS), float(BN), ml_dtypes.bfloat16)
    dlr = np.full((NCORES, 1, plan.TOTNS * P), float(BN), ml_dtypes.bfloat16)
    selfdl = np.minimum(np.arange(P), BN).astype(np.float64)
    for g in range(NGB):
        c, b = divmod(g, NBLK)
        n0 = g * BN
        fi = np.zeros(int(gsub[b]) * P, np.int64)
        fd = np.full(int(nsub[b]) * P, float(BN), np.float64)
        (sgl, dgl), (sgh, dgh) = lo[g], hi[g]
        nl, nh = len(sgl), len(sgh)
        fi[:nl] = sgl
        fd[:nl] = (dgl - n0).astype(np.float64)
        ob = int(gl[b]) * P
        fi[ob:ob + nh] = sgh
        fd[ob:ob + nh] = (dgh - n0).astype(np.float64)
        fd[int(gsub[b]) * P:] = selfdl
        idxm[c, :, plan.off8[b]:plan.off8[b + 1]] = _wrap16(fi)
        dlm[c, :, plan.offn[b]:plan.offn[b + 1]] = \
            fd.reshape(-1, P).T.astype(ml_dtypes.bfloat16)
        dlr[c, 0, plan.offn[b] * P:plan.offn[b + 1] * P] = fd.astype(ml_dtypes.bfloat16)
    _PLAN = plan
    return plan, 0, idxm, dlm, dlr


def _bcast(v, rows=P):
    v = np.asarray(v, np.float32).reshape(-1)
    return np.tile(v[None, :], (rows, 1)).astype(np.float32)


def _build(plan, _unused=0):
    MAXNS = plan.MAXNS
    nc = bacc.Bacc(num_swdge_queues=NQ)

    # ---- I/O ----
    xT_ext = nc.declare_dram_parameter("xT", [IN, NPC], F32, isOutput=False)
    idxm_ext = nc.declare_dram_parameter("idxm", [P, plan.TOTIDX8], I16, isOutput=False)
    dlm_ext = nc.declare_dram_parameter("dlm", [P, plan.TOTNS], BF16, isOutput=False)
    dlr_ext = nc.declare_dram_parameter("dlr", [1, plan.TOTNS * P], BF16, isOutput=False)
    win_ext = nc.declare_dram_parameter("win", [IN, HID], F32, isOutput=False)
    binb_ext = nc.declare_dram_parameter("binb", [P, HID], F32, isOutput=False)
    wl_ext, wr_ext, blb_ext, brb_ext, attr_ext, bob_ext, gb_ext, beb_ext = [], [], [], [], [], [], [], []
    for i in range(LAYERS):
        ic = HID if i == 0 else HC
        wl_ext.append(nc.declare_dram_parameter(f"wl{i}", [ic, HC], F32, isOutput=False))
        wr_ext.append(nc.declare_dram_parameter(f"wr{i}", [ic, HC], F32, isOutput=False))
        blb_ext.append(nc.declare_dram_parameter(f"blb{i}", [P, HC], F32, isOutput=False))
        brb_ext.append(nc.declare_dram_parameter(f"brb{i}", [P, HC], F32, isOutput=False))
        attr_ext.append(nc.declare_dram_parameter(f"attr{i}", [P, MAXNS * HC], BF16, isOutput=False))
        bob_ext.append(nc.declare_dram_parameter(f"bob{i}", [P, HC], F32, isOutput=False))
        gb_ext.append(nc.declare_dram_parameter(f"gb{i}", [P, HC], F32, isOutput=False))
        beb_ext.append(nc.declare_dram_parameter(f"beb{i}", [P, HC], F32, isOutput=False))
    wout_ext = nc.declare_dram_parameter("wout", [HC, EMB], F32, isOutput=False)
    boutb_ext = nc.declare_dram_parameter("boutb", [P, EMB], F32, isOutput=False)
    out_ext = nc.declare_dram_parameter("out", [NPC, EMB], F32, isOutput=True)

    with tile.TileContext(nc) as tc:
        with (
            tc.tile_pool(name="dram", bufs=1, space="DRAM") as dpool,
            tc.tile_pool(name="pers", bufs=1) as pers,
            tc.tile_pool(name="wpool", bufs=1) as wpool,
            tc.tile_pool(name="work", bufs=2) as work,
            tc.tile_pool(name="gbuf", bufs=3) as gbuf,
            tc.tile_pool(name="small", bufs=2) as small,
            tc.tile_pool(name="psA", bufs=2, space="PSUM") as psA,
            tc.tile_pool(name="psX", bufs=2, space="PSUM") as psX,
            tc.tile_pool(name="psB", bufs=2, space="PSUM") as psB,
        ):
            # ---- DRAM internals ----
            ag_in = [dpool.tile([NPC, HC], BF16, tag=f"ag_in{j}", name=f"ag_in{j}")
                     for j in range(LAYERS)]
            ag_out = [dpool.tile([N, HC], BF16, tag=f"ag_out{j}", name=f"ag_out{j}",
                                 addr_space="Shared") for j in range(LAYERS)]

            # ---- persistent SBUF ----
            hT_a = pers.tile([P, NPC], F32, tag="hT_a")
            hT_b = pers.tile([P, NPC], F32, tag="hT_b")
            xl_all = pers.tile([P, NBLK, HC], BF16, tag="xl_all")
            xr_own = pers.tile([P, NBLK, HC], BF16, tag="xr_own")
            iota_t = pers.tile([P, MAXNS, P], BF16, tag="iota")
            iotaP = pers.tile([P, MAXNS * P], BF16, tag="iotaP")
            attr_t = pers.tile([P, MAXNS * HC], BF16, tag="attr")
            id_t = pers.tile([P, P], F32, tag="ident")
            id_bf = pers.tile([P, P], BF16, tag="idbf")
            eps5_t = pers.tile([P, 1], F32, tag="eps5")
            acc_all = pers.tile([P, NBLK, HC + HEADS], F32, tag="acc_all")

            from concourse.masks import make_identity
            make_identity(nc, id_t[:])
            nc.vector.tensor_copy(id_bf[:], id_t[:])
            nc.vector.memset(eps5_t[:], 1e-5)
            nc.vector.memset(xl_all[:], 0.0)
            nc.vector.memset(xr_own[:], 0.0)
            ioi_t = pers.tile([P, MAXNS, P], mybir.dt.int32, tag="t_all", name="ioi_t")
            nc.gpsimd.iota(ioi_t[:], pattern=[[0, MAXNS], [1, P]], base=0, channel_multiplier=0)
            nc.vector.tensor_copy(iota_t[:], ioi_t[:])
            iop_t = pers.tile([P, MAXNS * P], mybir.dt.int32, tag="t_all", name="iop_t")
            nc.gpsimd.iota(iop_t[:], pattern=[[0, MAXNS * P]], base=0, channel_multiplier=1)
            nc.vector.tensor_copy(iotaP[:], iop_t[:])

            # ================= h0 = gelu(x @ W_in + b_in) =================
            win_t = wpool.tile([IN, HID], F32, tag="win")
            binb_t = wpool.tile([P, HID], F32, tag="binb")
            nc.sync.dma_start(win_t[:], win_ext[:])
            nc.sync.dma_start(binb_t[:], binb_ext[:])
            for b in range(NBLK):
                cs = slice(b * BN, (b + 1) * BN)
                xTb = work.tile([P, BN], F32, tag="xTb")
                nc.sync.dma_start(xTb[:IN, :], xT_ext[:, cs])
                ps = psA.tile([P, HC], F32, tag="mm")
                nc.tensor.matmul(ps[:BN, :HID], xTb[:IN, :], win_t[:], start=True, stop=True)
                h0s = work.tile([P, HID], F32, tag="h0s")
                nc.vector.tensor_tensor(out=h0s[:BN, :], in0=ps[:BN, :HID], in1=binb_t[:BN, :], op=OP.add)
                h0g = work.tile([P, HID], F32, tag="h0g")
                nc.scalar.activation(h0g[:BN, :], h0s[:BN, :], AF.Gelu)
                tp = psA.tile([HC, P], F32, tag="tp")
                nc.tensor.transpose(tp[:HID, :BN], h0g[:BN, :], id_t[:BN, :BN])
                nc.vector.tensor_copy(hT_a[:HID, cs], tp[:HID, :BN])

            hT_prev, hT_new = hT_a, hT_b

            for li in range(LAYERS):
                ic = HID if li == 0 else HC
                agi = ag_in[li]
                ago = ag_out[li]

                wl_t = wpool.tile([HC, HC], F32, tag="wl")
                wr_t = wpool.tile([HC, HC], F32, tag="wr")
                blb_t = wpool.tile([P, HC], F32, tag="blb")
                brb_t = wpool.tile([P, HC], F32, tag="brb")
                bob_t = wpool.tile([P, HC], F32, tag="bob")
                gb_t = wpool.tile([P, HC], F32, tag="gb")
                beb_t = wpool.tile([P, HC], F32, tag="beb")
                nc.sync.dma_start(wl_t[:ic, :], wl_ext[li][:])
                nc.sync.dma_start(wr_t[:ic, :], wr_ext[li][:])
                nc.sync.dma_start(blb_t[:], blb_ext[li][:])
                nc.sync.dma_start(brb_t[:], brb_ext[li][:])
                nc.sync.dma_start(attr_t[:], attr_ext[li][:])
                nc.sync.dma_start(bob_t[:], bob_ext[li][:])
                nc.sync.dma_start(gb_t[:], gb_ext[li][:])
                nc.sync.dma_start(beb_t[:], beb_ext[li][:])

                # ---- xr (own nodes) -> SBUF xr_own + DRAM ag_in, then AllGather ----
                for b in range(NBLK):
                    cs = slice(b * BN, (b + 1) * BN)
                    ps = psA.tile([P, HC], F32, tag="mm")
                    nc.tensor.matmul(ps[:BN, :], hT_prev[:ic, cs], wr_t[:ic, :], start=True, stop=True)
                    nc.vector.tensor_tensor(out=xr_own[:BN, b, :], in0=ps[:BN, :], in1=brb_t[:BN, :], op=OP.add)
                nc.sync.dma_start(
                    agi[:].rearrange("(b n) c -> n b c", b=NBLK), xr_own[:BN, :, :])
                nc.gpsimd.collective_compute(
                    "AllGather", OP.bypass, replica_groups=[list(range(NCORES))],
                    ins=[agi.opt()], outs=[ago.opt()],
                )

                # ---- xl (own nodes), overlaps the collective ----
                for b in range(NBLK):
                    cs = slice(b * BN, (b + 1) * BN)
                    ps = psA.tile([P, HC], F32, tag="mm")
                    nc.tensor.matmul(ps[:BN, :], hT_prev[:ic, cs], wl_t[:ic, :], start=True, stop=True)
                    nc.vector.tensor_tensor(out=xl_all[:BN, b, :], in0=ps[:BN, :], in1=blb_t[:BN, :], op=OP.add)

                # ---- layernorm + gelu + residual postlude over acc_all ----
                def _post(b0, b1, li=li, bob_t=bob_t, gb_t=gb_t, beb_t=beb_t,
                          hT_prev=hT_prev, hT_new=hT_new):
                    HB = b1 - b0
                    t_ap = acc_all[:BN, b0:b1, :HC]
                    den_t = small.tile([P, NBLK, HEADS], F32, tag="den", name="den_t")
                    nc.vector.tensor_scalar(out=den_t[:BN, :HB, :], in0=acc_all[:BN, b0:b1, HC:],
                                            scalar1=1e-16, scalar2=None, op0=OP.add)
                    rec_t = small.tile([P, NBLK, HEADS], F32, tag="rec", name="rec_t")
                    nc.vector.reciprocal(rec_t[:BN, :HB, :], den_t[:BN, :HB, :])
                    nc.vector.tensor_tensor(
                        out=t_ap.rearrange("p b (h c) -> p b h c", h=HEADS),
                        in0=t_ap.rearrange("p b (h c) -> p b h c", h=HEADS),
                        in1=rec_t[:BN, :HB, :, None].broadcast_to([BN, HB, HEADS, CH]), op=OP.mult)
                    nc.vector.tensor_tensor(
                        out=t_ap, in0=t_ap,
                        in1=bob_t[:BN, None, :].broadcast_to([BN, HB, HC]), op=OP.add)
                    mu_t = small.tile([P, NBLK], F32, tag="mu", name="mu_t")
                    nc.vector.reduce_sum(mu_t[:BN, :HB], t_ap, axis=mybir.AxisListType.X)
                    nc.vector.tensor_scalar(out=mu_t[:BN, :HB], in0=mu_t[:BN, :HB],
                                            scalar1=1.0 / HC, scalar2=None, op0=OP.mult)
                    nc.vector.tensor_tensor(
                        out=t_ap, in0=t_ap,
                        in1=mu_t[:BN, :HB, None].broadcast_to([BN, HB, HC]), op=OP.subtract)
                    var_t = small.tile([P, NBLK], F32, tag="var", name="var_t")
                    sqs_t = small.tile([P, HC], F32, tag="sqs", name="sqs_t")
                    for b in range(b0, b1):
                        nc.scalar.activation(sqs_t[:BN, :], acc_all[:BN, b, :HC], AF.Square,
                                             accum_out=var_t[:BN, b - b0:b - b0 + 1])
                    std_t = small.tile([P, NBLK], F32, tag="std", name="std_t")
                    nc.scalar.activation(std_t[:BN, :HB], var_t[:BN, :HB], AF.Sqrt,
                                         scale=1.0 / HC, bias=eps5_t[:BN, :1])
                    rstd_t = small.tile([P, NBLK], F32, tag="rstd", name="rstd_t")
                    nc.vector.reciprocal(rstd_t[:BN, :HB], std_t[:BN, :HB])
                    nc.vector.tensor_tensor(
                        out=t_ap, in0=t_ap,
                        in1=rstd_t[:BN, :HB, None].broadcast_to([BN, HB, HC]), op=OP.mult)
                    nc.vector.tensor_tensor(
                        out=t_ap, in0=t_ap,
                        in1=gb_t[:BN, None, :].broadcast_to([BN, HB, HC]), op=OP.mult)
                    nc.vector.tensor_tensor(
                        out=t_ap, in0=t_ap,
                        in1=beb_t[:BN, None, :].broadcast_to([BN, HB, HC]), op=OP.add)
                    nc.scalar.activation(t_ap, t_ap, AF.Gelu)
                    for b in range(b0, b1):
                        cs = slice(b * BN, (b + 1) * BN)
                        tp = psA.tile([HC, P], F32, tag="tp")
                        nc.tensor.transpose(tp[:, :BN], acc_all[:BN, b, :HC], id_t[:BN, :BN])
                        if li == 0:
                            nc.vector.tensor_copy(hT_new[:, cs], tp[:, :BN])
                        else:
                            nc.vector.tensor_tensor(out=hT_new[:, cs], in0=tp[:, :BN],
                                                    in1=hT_prev[:, cs], op=OP.add)

                # ---- edge blocks ----
                for b in range(NBLK):
                    if b >= 13 and (b - 13) % 10 == 0 and b < 50:
                        pc = (b - 13) // 10
                        _post(pc * 10, pc * 10 + 10)
                    ns = int(plan.nsub[b])
                    gs = int(plan.gsub[b])
                    glb = int(plan.gl[b])
                    ghb = int(plan.gh[b])
                    o8 = int(plan.off8[b])
                    on = int(plan.offn[b])

                    idxm_t = small.tile([P, 8 * MAXNS], I16, tag="idxm", bufs=3)
                    dlm_t = small.tile([P, MAXNS], BF16, tag="dlm", bufs=3)
                    dstb_t = work.tile([P, MAXNS * P], BF16, tag="dstb")
                    nc.sync.dma_start(idxm_t[:, :8 * gs], idxm_ext[:, o8:o8 + 8 * gs])
                    nc.sync.dma_start(dlm_t[:, :ns], dlm_ext[:, on:on + ns])
                    nc.sync.dma_start(dstb_t[:, :ns * P],
                                      dlr_ext[:1, on * P:(on + ns) * P].broadcast_to([P, ns * P]))

                    xj_t = gbuf.tile([P, MAXNS, HC], BF16, tag="xj")
                    if glb:
                        nc.gpsimd.dma_gather(
                            out_ap=xj_t[:, :glb, :], in_ap=ago[:SPLIT, :],
                            idxs_ap=idxm_t[:, :glb * 8],
                            num_idxs=glb * P, num_idxs_reg=glb * P, elem_size=HC,
                            single_packet=False, queue_num=(2 * b) % NQ)
                    if ghb:
                        nc.gpsimd.dma_gather(
                            out_ap=xj_t[:, glb:gs, :], in_ap=ago[SPLIT:, :],
                            idxs_ap=idxm_t[:, glb * 8:gs * 8],
                            num_idxs=ghb * P, num_idxs_reg=ghb * P, elem_size=HC,
                            single_packet=False, queue_num=(2 * b + 1) % NQ)

                    st_t = gbuf.tile([P, MAXNS, P], BF16, tag="st")
                    nc.vector.tensor_tensor(
                        out=st_t[:, :ns, :], in0=iota_t[:, :ns, :],
                        in1=dlm_t[:, :ns, None].broadcast_to([P, ns, P]), op=OP.is_equal)
                    s_t = work.tile([P, MAXNS * P], BF16, tag="s_t")
                    nc.vector.tensor_tensor(
                        out=s_t[:, :ns * P], in0=dstb_t[:, :ns * P], in1=iotaP[:, :ns * P],
                        op=OP.is_equal)

                    # x_i expansion + x_j add on PE, prelu from PSUM on Scalar
                    et_sep = work.tile([P, MAXNS, HC], BF16, tag="etsep", name="et_sep")
                    et_t = et_sep[:, :, :]
                    _peadd = _os.environ.get("V3PEADD", "1") == "1"
                    nc.vector.tensor_copy(xj_t[:, gs, :], xr_own[:, b, :])
                    j0 = 0
                    while j0 < ns:
                        j1 = min(ns, j0 + ETCH)
                        w = j1 - j0
                        psE = psX.tile([P, ETCH * HC], F32, tag="xi")
                        if _peadd:
                            nc.tensor.matmul(
                                psE[:, :w * HC], id_bf[:],
                                xj_t[:, j0:j1, :].rearrange("p a c -> p (a c)"),
                                start=True, stop=False, skip_group_check=True)
                        for j in range(j0, j1):
                            nc.tensor.matmul(
                                psE[:, (j - j0) * HC:(j - j0 + 1) * HC],
                                s_t[:, j * P:(j + 1) * P], xl_all[:, b, :],
                                start=(not _peadd), stop=((not _peadd) or j == j1 - 1),
                                skip_group_check=True)
                        if _peadd:
                            nc.scalar.activation(
                                et_t[:, j0:j1, :],
                                psE[:, :w * HC].rearrange("p (a c) -> p a c", c=HC),
                                AF.Prelu, alpha=NEG)
                        else:
                            nc.vector.tensor_tensor(
                                out=et_t[:, j0:j1, :],
                                in0=psE[:, :w * HC].rearrange("p (a c) -> p a c", c=HC),
                                in1=xj_t[:, j0:j1, :], op=OP.add)
                            nc.scalar.activation(
                                et_t[:, j0:j1, :], et_t[:, j0:j1, :],
                                AF.Prelu, alpha=NEG)
                        j0 = j1

                    nc.vector.tensor_tensor(
                        out=et_t[:, :ns, :],
                        in0=et_t[:, :ns, :],
                        in1=attr_t[:, :ns * HC].rearrange("p (a c) -> p a c", c=HC), op=OP.mult)
                    lg_t = small.tile([P, MAXNS, HEADS], F32, tag="lg")
                    nc.vector.reduce_sum(
                        lg_t[:, :ns, :], et_t[:, :ns, :].rearrange("p j (h c) -> p j h c", h=HEADS),
                        axis=mybir.AxisListType.X)
                    v_t = work.tile([P, MAXNS, HC + HEADS], BF16, tag="vt", name="v_t")
                    nc.scalar.activation(v_t[:, :ns, HC:], lg_t[:, :ns, :], AF.Exp)
                    nc.vector.tensor_tensor(
                        out=v_t[:, :ns, :HC].rearrange("p j (h c) -> p j h c", h=HEADS),
                        in0=xj_t[:, :ns, :].rearrange("p j (h c) -> p j h c", h=HEADS),
                        in1=v_t[:, :ns, HC:, None].broadcast_to([P, ns, HEADS, CH]), op=OP.mult)

                    acc = psB.tile([P, HC + HEADS], F32, tag="acc")
                    for j in range(ns):
                        nc.tensor.matmul(acc[:], st_t[:, j, :], v_t[:, j, :],
                                         start=(j == 0), stop=(j == ns - 1))
                    nc.vector.tensor_copy(acc_all[:BN, b, :], acc[:BN, :])

                _post(40, NBLK)

                hT_prev, hT_new = hT_new, hT_prev

            # ================= out = normalize(h @ W_out + b_out) =================
            wout_t = wpool.tile([HC, EMB], F32, tag="wout")
            boutb_t = wpool.tile([P, EMB], F32, tag="boutb")
            nc.sync.dma_start(wout_t[:], wout_ext[:])
            nc.sync.dma_start(boutb_t[:], boutb_ext[:])
            for b in range(NBLK):
                cs = slice(b * BN, (b + 1) * BN)
                ps = psA.tile([P, EMB], F32, tag="mm")
                nc.tensor.matmul(ps[:BN, :], hT_prev[:HC, cs], wout_t[:], start=True, stop=True)
                osb = work.tile([P, EMB], F32, tag="osb")
                nc.vector.tensor_tensor(out=osb[:BN, :], in0=ps[:BN, :], in1=boutb_t[:BN, :], op=OP.add)
                sq_t = work.tile([P, EMB], F32, tag="osq")
                nsq_t = small.tile([P, 1], F32, tag="nsq")
                nc.scalar.activation(sq_t[:BN, :], osb[:BN, :], AF.Square, accum_out=nsq_t[:BN, :1])
                nrm_t = small.tile([P, 1], F32, tag="nrm")
                nc.scalar.activation(nrm_t[:BN, :], nsq_t[:BN, :], AF.Sqrt)
                nc.vector.tensor_scalar(out=nrm_t[:BN, :], in0=nrm_t[:BN, :], scalar1=1e-12,
                                        scalar2=None, op0=OP.max)
                recn_t = small.tile([P, 1], F32, tag="recn")
                nc.vector.reciprocal(recn_t[:BN, :], nrm_t[:BN, :])
                nc.vector.tensor_scalar(out=osb[:BN, :], in0=osb[:BN, :], scalar1=recn_t[:BN, :1],
                                        scalar2=None, op0=OP.mult)
                nc.sync.dma_start(out_ext[cs, :], osb[:BN, :])

    nc.compile()
    return nc


def _make_in_maps(inputs, idxm, dlm, dlr):
    plan = _PLAN
    x = np.asarray(inputs["x"], np.float32)
    common = {
        "win": np.asarray(inputs["W_in"], np.float32),
        "binb": _bcast(inputs["b_in"]),
        "wout": np.asarray(inputs["W_out"], np.float32),
        "boutb": _bcast(inputs["b_out"]),
    }
    for i in range(LAYERS):
        common[f"wl{i}"] = np.asarray(inputs[f"Wl{i}"], np.float32)
        common[f"wr{i}"] = np.asarray(inputs[f"Wr{i}"], np.float32)
        common[f"blb{i}"] = _bcast(inputs[f"bl{i}"])
        common[f"brb{i}"] = _bcast(inputs[f"br{i}"])
        att = np.asarray(inputs[f"att{i}"], np.float32).reshape(-1)
        common[f"attr{i}"] = np.tile(att[None, :], (P, plan.MAXNS)).astype(ml_dtypes.bfloat16)
        common[f"bob{i}"] = _bcast(inputs[f"bo{i}"])
        common[f"gb{i}"] = _bcast(inputs[f"g{i}"])
        common[f"beb{i}"] = _bcast(inputs[f"be{i}"])
    in_maps = []
    for c in range(NCORES):
        m = dict(common)
        m["xT"] = np.ascontiguousarray(x[c * NPC:(c + 1) * NPC, :].T)
        m["idxm"] = idxm[c]
        m["dlm"] = dlm[c]
        m["dlr"] = dlr[c]
        in_maps.append(m)
    return in_maps


def kernel(**inputs):
    edge_index = np.asarray(inputs["edge_index"])
    plan, _z, idxm, dlm, dlr = _preprocess(edge_index)
    nc = _build(plan)
    in_maps = _make_in_maps(inputs, idxm, dlm, dlr)
    res = run_bass_kernel_spmd(nc, in_maps, list(range(NCORES)))
    out = np.concatenate([res.results[c]["out"] for c in range(NCORES)], axis=0)
    return out.astype(np.float32)


# revision 18
# speedup vs baseline: 1.0511x; 1.0511x over previous
"""Trainium2 Bass kernel for nn_CardGNN (3-layer GATv2 message passing), v3.

Sharding: nodes partitioned across 8 NeuronCores (6250 each, 50 blocks of 125
dst nodes). Each core computes xr = h @ Wr for its OWN nodes only and the
node-major bf16 table is AllGathered (replaces v2's 8x-redundant table
compute). Per-edge source features are fetched with dma_gather (bulk 256B
rows, int16 indices, table split at row 32750), round-robining 4 SWDGE
queues. Self-loop features come from an SBUF-resident copy (no gather, and
exact per-block-slot subtile counts instead of a global max). The x_i one-hot
expansion and the x_i+x_j add both run on the TensorEngine accumulating in
PSUM; leaky-relu reads PSUM on the Scalar engine. Segment softmax +
aggregation are one-hot matmuls accumulating in PSUM.
"""
import math
import numpy as np
import ml_dtypes

import concourse.bacc as bacc
import concourse.mybir as mybir
import concourse.tile as tile
from concourse.bass_utils import run_bass_kernel_spmd

F32 = mybir.dt.float32
BF16 = mybir.dt.bfloat16
I16 = mybir.dt.int16
AF = mybir.ActivationFunctionType
OP = mybir.AluOpType

N = 50000
E = 800000
IN = 128
HID = 32
HEADS = 4
CH = 32
HC = HID * HEADS  # 128
EMB = 128
NCORES = 8
NPC = N // NCORES          # 6250 nodes per core
BN = 125                   # dst nodes per block
NBLK = NPC // BN           # 50 blocks per core
NGB = N // BN              # 400 global blocks
P = 128
SPLIT = 32750              # lo/hi table split so idx fits int16
NEG = 0.2
LAYERS = 3
import os as _os
NQ = int(_os.environ.get("V3NQ", "4"))  # SWDGE queues
ETCH = 4                   # et psum chunk, subtiles

_PLAN = None


class Plan:
    pass


def _wrap16(idx_flat):
    """dma_gather index layout: w[p, s] = idx[s*16+p], replicated to 128 rows."""
    w = idx_flat.reshape(-1, 16).T.astype(np.int16)
    return np.tile(w, (8, 1))


def _preprocess(edge_index):
    global _PLAN
    src = np.asarray(edge_index[0]).astype(np.int64)
    dst = np.asarray(edge_index[1]).astype(np.int64)
    order = np.argsort(dst, kind="stable")
    src = src[order]
    dst = dst[order]
    blk = dst // BN
    starts = np.searchsorted(blk, np.arange(NGB))
    ends = np.searchsorted(blk, np.arange(NGB) + 1)

    lo, hi = [], []
    for g in range(NGB):
        s, e = starts[g], ends[g]
        sg, dg = src[s:e], dst[s:e]
        m = sg < SPLIT
        lo.append((sg[m], dg[m]))
        hi.append((sg[~m] - SPLIT, dg[~m]))

    gl = np.zeros(NBLK, np.int64)
    gh = np.zeros(NBLK, np.int64)
    for g in range(NGB):
        b = g % NBLK
        gl[b] = max(gl[b], math.ceil(len(lo[g][0]) / P))
        gh[b] = max(gh[b], math.ceil(len(hi[g][0]) / P))
    gsub = gl + gh
    nsub = gsub + 1          # + self subtile

    plan = Plan()
    plan.gl, plan.gh, plan.gsub, plan.nsub = gl, gh, gsub, nsub
    plan.off8 = np.concatenate([[0], np.cumsum(8 * gsub)])
    plan.offn = np.concatenate([[0], np.cumsum(nsub)])
    plan.TOTIDX8 = int(plan.off8[-1])
    plan.TOTNS = int(plan.offn[-1])
    plan.MAXNS = int(nsub.max())

    idxm = np.zeros((NCORES, P, plan.TOTIDX8), np.int16)
    dlm = np.full((NCORES, P, plan.TOTNS), float(BN), ml_dtypes.bfloat16)
    dlr = np.full((NCORES, 1, plan.TOTNS * P), float(BN), ml_dtypes.bfloat16)
    selfdl = np.minimum(np.arange(P), BN).astype(np.float64)
    for g in range(NGB):
        c, b = divmod(g, NBLK)
        n0 = g * BN
        fi = np.zeros(int(gsub[b]) * P, np.int64)
        fd = np.full(int(nsub[b]) * P, float(BN), np.float64)
        (sgl, dgl), (sgh, dgh) = lo[g], hi[g]
        nl, nh = len(sgl), len(sgh)
        fi[:nl] = sgl
        fd[:nl] = (dgl - n0).astype(np.float64)
        ob = int(gl[b]) * P
        fi[ob:ob + nh] = sgh
        fd[ob:ob + nh] = (dgh - n0).astype(np.float64)
        fd[int(gsub[b]) * P:] = selfdl
        idxm[c, :, plan.off8[b]:plan.off8[b + 1]] = _wrap16(fi)
        dlm[c, :, plan.offn[b]:plan.offn[b + 1]] = \
            fd.reshape(-1, P).T.astype(ml_dtypes.bfloat16)
        dlr[c, 0, plan.offn[b] * P:plan.offn[b + 1] * P] = fd.astype(ml_dtypes.bfloat16)
    _PLAN = plan
    return plan, 0, idxm, dlm, dlr


def _bcast(v, rows=P):
    v = np.asarray(v, np.float32).reshape(-1)
    return np.tile(v[None, :], (rows, 1)).astype(np.float32)


def _build(plan, _unused=0):
    MAXNS = plan.MAXNS
    nc = bacc.Bacc(num_swdge_queues=NQ)

    # ---- I/O ----
    xT_ext = nc.declare_dram_parameter("xT", [IN, NPC], F32, isOutput=False)
    idxm_ext = nc.declare_dram_parameter("idxm", [P, plan.TOTIDX8], I16, isOutput=False)
    dlm_ext = nc.declare_dram_parameter("dlm", [P, plan.TOTNS], BF16, isOutput=False)
    dlr_ext = nc.declare_dram_parameter("dlr", [1, plan.TOTNS * P], BF16, isOutput=False)
    win_ext = nc.declare_dram_parameter("win", [IN, HID], F32, isOutput=False)
    binb_ext = nc.declare_dram_parameter("binb", [P, HID], F32, isOutput=False)
    wl_ext, wr_ext, blb_ext, brb_ext, attr_ext, bob_ext, gb_ext, beb_ext = [], [], [], [], [], [], [], []
    for i in range(LAYERS):
        ic = HID if i == 0 else HC
        wl_ext.append(nc.declare_dram_parameter(f"wl{i}", [ic, HC], F32, isOutput=False))
        wr_ext.append(nc.declare_dram_parameter(f"wr{i}", [ic, HC], F32, isOutput=False))
        blb_ext.append(nc.declare_dram_parameter(f"blb{i}", [P, HC], F32, isOutput=False))
        brb_ext.append(nc.declare_dram_parameter(f"brb{i}", [P, HC], F32, isOutput=False))
        attr_ext.append(nc.declare_dram_parameter(f"attr{i}", [P, MAXNS * HC], BF16, isOutput=False))
        bob_ext.append(nc.declare_dram_parameter(f"bob{i}", [P, HC], F32, isOutput=False))
        gb_ext.append(nc.declare_dram_parameter(f"gb{i}", [P, HC], F32, isOutput=False))
        beb_ext.append(nc.declare_dram_parameter(f"beb{i}", [P, HC], F32, isOutput=False))
    wout_ext = nc.declare_dram_parameter("wout", [HC, EMB], F32, isOutput=False)
    boutb_ext = nc.declare_dram_parameter("boutb", [P, EMB], F32, isOutput=False)
    out_ext = nc.declare_dram_parameter("out", [NPC, EMB], F32, isOutput=True)

    with tile.TileContext(nc) as tc:
        with (
            tc.tile_pool(name="dram", bufs=1, space="DRAM") as dpool,
            tc.tile_pool(name="pers", bufs=1) as pers,
            tc.tile_pool(name="wpool", bufs=1) as wpool,
            tc.tile_pool(name="work", bufs=2) as work,
            tc.tile_pool(name="gbuf", bufs=4) as gbuf,
            tc.tile_pool(name="small", bufs=2) as small,
            tc.tile_pool(name="psA", bufs=2, space="PSUM") as psA,
            tc.tile_pool(name="psX", bufs=2, space="PSUM") as psX,
            tc.tile_pool(name="psB", bufs=2, space="PSUM") as psB,
        ):
            # ---- DRAM internals ----
            ag_in = [dpool.tile([NPC, HC], BF16, tag=f"ag_in{j}", name=f"ag_in{j}")
                     for j in range(LAYERS)]
            ag_out = [dpool.tile([N, HC], BF16, tag=f"ag_out{j}", name=f"ag_out{j}",
                                 addr_space="Shared") for j in range(LAYERS)]

            # ---- persistent SBUF ----
            hT_a = pers.tile([P, NPC], F32, tag="hT_a")
            hT_b = pers.tile([P, NPC], F32, tag="hT_b")
            xl_all = pers.tile([P, NBLK, HC], BF16, tag="xl_all")
            xr_own = pers.tile([P, NBLK, HC], BF16, tag="xr_own")
            iota_t = pers.tile([P, MAXNS, P], BF16, tag="iota")
            iotaP = pers.tile([P, MAXNS * P], BF16, tag="iotaP")
            attr_t = pers.tile([P, MAXNS * HC], BF16, tag="attr")
            id_t = pers.tile([P, P], F32, tag="ident")
            id_bf = pers.tile([P, P], BF16, tag="idbf")
            eps5_t = pers.tile([P, 1], F32, tag="eps5")
            acc_all = pers.tile([P, NBLK, HC + HEADS], F32, tag="acc_all")

            from concourse.masks import make_identity
            make_identity(nc, id_t[:])
            nc.vector.tensor_copy(id_bf[:], id_t[:])
            nc.vector.memset(eps5_t[:], 1e-5)
            nc.vector.memset(xl_all[:], 0.0)
            nc.vector.memset(xr_own[:], 0.0)
            ioi_t = pers.tile([P, MAXNS, P], mybir.dt.int32, tag="hT_b", name="ioi_t")
            nc.gpsimd.iota(ioi_t[:], pattern=[[0, MAXNS], [1, P]], base=0, channel_multiplier=0)
            nc.vector.tensor_copy(iota_t[:], ioi_t[:])
            iop_t = pers.tile([P, MAXNS * P], mybir.dt.int32, tag="hT_b", name="iop_t")
            nc.gpsimd.iota(iop_t[:], pattern=[[0, MAXNS * P]], base=0, channel_multiplier=1)
            nc.vector.tensor_copy(iotaP[:], iop_t[:])

            # ================= h0 = gelu(x @ W_in + b_in) =================
            win_t = wpool.tile([IN, HID], F32, tag="win")
            binb_t = wpool.tile([P, HID], F32, tag="binb")
            nc.sync.dma_start(win_t[:], win_ext[:])
            nc.sync.dma_start(binb_t[:], binb_ext[:])
            for b in range(NBLK):
                cs = slice(b * BN, (b + 1) * BN)
                xTb = work.tile([P, BN], F32, tag="xTb")
                nc.sync.dma_start(xTb[:IN, :], xT_ext[:, cs])
                ps = psA.tile([P, HC], F32, tag="mm")
                nc.tensor.matmul(ps[:BN, :HID], xTb[:IN, :], win_t[:], start=True, stop=True)
                h0s = work.tile([P, HID], F32, tag="h0s")
                nc.vector.tensor_tensor(out=h0s[:BN, :], in0=ps[:BN, :HID], in1=binb_t[:BN, :], op=OP.add)
                h0g = work.tile([P, HID], F32, tag="h0g")
                nc.scalar.activation(h0g[:BN, :], h0s[:BN, :], AF.Gelu)
                tp = psA.tile([HC, P], F32, tag="tp")
                nc.tensor.transpose(tp[:HID, :BN], h0g[:BN, :], id_t[:BN, :BN])
                nc.vector.tensor_copy(hT_a[:HID, cs], tp[:HID, :BN])

            hT_prev, hT_new = hT_a, hT_b

            for li in range(LAYERS):
                ic = HID if li == 0 else HC
                agi = ag_in[li]
                ago = ag_out[li]

                wl_t = wpool.tile([HC, HC], F32, tag="wl")
                wr_t = wpool.tile([HC, HC], F32, tag="wr")
                blb_t = wpool.tile([P, HC], F32, tag="blb")
                brb_t = wpool.tile([P, HC], F32, tag="brb")
                bob_t = wpool.tile([P, HC], F32, tag="bob")
                gb_t = wpool.tile([P, HC], F32, tag="gb")
                beb_t = wpool.tile([P, HC], F32, tag="beb")
                nc.sync.dma_start(wl_t[:ic, :], wl_ext[li][:])
                nc.sync.dma_start(wr_t[:ic, :], wr_ext[li][:])
                nc.sync.dma_start(blb_t[:], blb_ext[li][:])
                nc.sync.dma_start(brb_t[:], brb_ext[li][:])
                nc.sync.dma_start(attr_t[:], attr_ext[li][:])
                nc.sync.dma_start(bob_t[:], bob_ext[li][:])
                nc.sync.dma_start(gb_t[:], gb_ext[li][:])
                nc.sync.dma_start(beb_t[:], beb_ext[li][:])

                # ---- xr (own nodes) -> SBUF xr_own + DRAM ag_in, then AllGather ----
                for b in range(NBLK):
                    cs = slice(b * BN, (b + 1) * BN)
                    ps = psA.tile([P, HC], F32, tag="mm")
                    nc.tensor.matmul(ps[:BN, :], hT_prev[:ic, cs], wr_t[:ic, :], start=True, stop=True)
                    nc.vector.tensor_tensor(out=xr_own[:BN, b, :], in0=ps[:BN, :], in1=brb_t[:BN, :], op=OP.add)
                nc.sync.dma_start(
                    agi[:].rearrange("(b n) c -> n b c", b=NBLK), xr_own[:BN, :, :])
                nc.gpsimd.collective_compute(
                    "AllGather", OP.bypass, replica_groups=[list(range(NCORES))],
                    ins=[agi.opt()], outs=[ago.opt()],
                )

                # ---- xl (own nodes), overlaps the collective ----
                for b in range(NBLK):
                    cs = slice(b * BN, (b + 1) * BN)
                    ps = psA.tile([P, HC], F32, tag="mm")
                    nc.tensor.matmul(ps[:BN, :], hT_prev[:ic, cs], wl_t[:ic, :], start=True, stop=True)
                    nc.vector.tensor_tensor(out=xl_all[:BN, b, :], in0=ps[:BN, :], in1=blb_t[:BN, :], op=OP.add)

                # ---- layernorm + gelu + residual postlude over acc_all ----
                def _post(b0, b1, li=li, bob_t=bob_t, gb_t=gb_t, beb_t=beb_t,
                          hT_prev=hT_prev, hT_new=hT_new):
                    HB = b1 - b0
                    t_ap = acc_all[:BN, b0:b1, :HC]
                    den_t = small.tile([P, NBLK, HEADS], F32, tag="den", name="den_t")
                    nc.vector.tensor_scalar(out=den_t[:BN, :HB, :], in0=acc_all[:BN, b0:b1, HC:],
                                            scalar1=1e-16, scalar2=None, op0=OP.add)
                    rec_t = small.tile([P, NBLK, HEADS], F32, tag="rec", name="rec_t")
                    nc.vector.reciprocal(rec_t[:BN, :HB, :], den_t[:BN, :HB, :])
                    nc.vector.tensor_tensor(
                        out=t_ap.rearrange("p b (h c) -> p b h c", h=HEADS),
                        in0=t_ap.rearrange("p b (h c) -> p b h c", h=HEADS),
                        in1=rec_t[:BN, :HB, :, None].broadcast_to([BN, HB, HEADS, CH]), op=OP.mult)
                    nc.vector.tensor_tensor(
                        out=t_ap, in0=t_ap,
                        in1=bob_t[:BN, None, :].broadcast_to([BN, HB, HC]), op=OP.add)
                    mu_t = small.tile([P, NBLK], F32, tag="mu", name="mu_t")
                    nc.vector.reduce_sum(mu_t[:BN, :HB], t_ap, axis=mybir.AxisListType.X)
                    nc.vector.tensor_scalar(out=mu_t[:BN, :HB], in0=mu_t[:BN, :HB],
                                            scalar1=1.0 / HC, scalar2=None, op0=OP.mult)
                    nc.vector.tensor_tensor(
                        out=t_ap, in0=t_ap,
                        in1=mu_t[:BN, :HB, None].broadcast_to([BN, HB, HC]), op=OP.subtract)
                    var_t = small.tile([P, NBLK], F32, tag="var", name="var_t")
                    sqs_t = small.tile([P, HC], F32, tag="sqs", name="sqs_t")
                    for b in range(b0, b1):
                        nc.scalar.activation(sqs_t[:BN, :], acc_all[:BN, b, :HC], AF.Square,
                                             accum_out=var_t[:BN, b - b0:b - b0 + 1])
                    std_t = small.tile([P, NBLK], F32, tag="std", name="std_t")
                    nc.scalar.activation(std_t[:BN, :HB], var_t[:BN, :HB], AF.Sqrt,
                                         scale=1.0 / HC, bias=eps5_t[:BN, :1])
                    rstd_t = small.tile([P, NBLK], F32, tag="rstd", name="rstd_t")
                    nc.vector.reciprocal(rstd_t[:BN, :HB], std_t[:BN, :HB])
                    nc.vector.tensor_tensor(
                        out=t_ap, in0=t_ap,
                        in1=rstd_t[:BN, :HB, None].broadcast_to([BN, HB, HC]), op=OP.mult)
                    nc.vector.tensor_tensor(
                        out=t_ap, in0=t_ap,
                        in1=gb_t[:BN, None, :].broadcast_to([BN, HB, HC]), op=OP.mult)
                    nc.vector.tensor_tensor(
                        out=t_ap, in0=t_ap,
                        in1=beb_t[:BN, None, :].broadcast_to([BN, HB, HC]), op=OP.add)
                    nc.scalar.activation(t_ap, t_ap, AF.Gelu)
                    for b in range(b0, b1):
                        cs = slice(b * BN, (b + 1) * BN)
                        tp = psA.tile([HC, P], F32, tag="tp")
                        nc.tensor.transpose(tp[:, :BN], acc_all[:BN, b, :HC], id_t[:BN, :BN])
                        if li == 0:
                            nc.vector.tensor_copy(hT_new[:, cs], tp[:, :BN])
                        else:
                            nc.vector.tensor_tensor(out=hT_new[:, cs], in0=tp[:, :BN],
                                                    in1=hT_prev[:, cs], op=OP.add)

                # ---- edge blocks ----
                for b in range(NBLK):
                    if b >= 13 and (b - 13) % 10 == 0 and b < 50:
                        pc = (b - 13) // 10
                        _post(pc * 10, pc * 10 + 10)
                    if b == 48:
                        _post(40, 45)
                    ns = int(plan.nsub[b])
                    gs = int(plan.gsub[b])
                    glb = int(plan.gl[b])
                    ghb = int(plan.gh[b])
                    o8 = int(plan.off8[b])
                    on = int(plan.offn[b])

                    idxm_t = small.tile([P, 8 * MAXNS], I16, tag="idxm", bufs=3)
                    dlm_t = small.tile([P, MAXNS], BF16, tag="dlm", bufs=3)
                    dstb_t = work.tile([P, MAXNS * P], BF16, tag="dstb")
                    nc.sync.dma_start(idxm_t[:, :8 * gs], idxm_ext[:, o8:o8 + 8 * gs])
                    nc.sync.dma_start(dlm_t[:, :ns], dlm_ext[:, on:on + ns])
                    nc.sync.dma_start(dstb_t[:, :gs * P],
                                      dlr_ext[:1, on * P:(on + gs) * P].broadcast_to([P, gs * P]))

                    xj_t = gbuf.tile([P, MAXNS, HC], BF16, tag="xj")
                    if glb:
                        nc.gpsimd.dma_gather(
                            out_ap=xj_t[:, :glb, :], in_ap=ago[:SPLIT, :],
                            idxs_ap=idxm_t[:, :glb * 8],
                            num_idxs=glb * P, num_idxs_reg=glb * P, elem_size=HC,
                            single_packet=False, queue_num=(2 * b) % NQ)
                    if ghb:
                        nc.gpsimd.dma_gather(
                            out_ap=xj_t[:, glb:gs, :], in_ap=ago[SPLIT:, :],
                            idxs_ap=idxm_t[:, glb * 8:gs * 8],
                            num_idxs=ghb * P, num_idxs_reg=ghb * P, elem_size=HC,
                            single_packet=False, queue_num=(2 * b + 1) % NQ)

                    st_t = gbuf.tile([P, MAXNS, P], BF16, tag="st")
                    nc.vector.tensor_tensor(
                        out=st_t[:, :ns, :], in0=iota_t[:, :ns, :],
                        in1=dlm_t[:, :ns, None].broadcast_to([P, ns, P]), op=OP.is_equal)
                    s_t = work.tile([P, MAXNS * P], BF16, tag="s_t")
                    nc.vector.tensor_tensor(
                        out=s_t[:, :gs * P], in0=dstb_t[:, :gs * P], in1=iotaP[:, :gs * P],
                        op=OP.is_equal)

                    # x_i expansion + x_j add on PE, prelu from PSUM on Scalar
                    et_sep = work.tile([P, MAXNS, HC], BF16, tag="etsep", name="et_sep")
                    et_t = et_sep[:, :, :]
                    _peadd = _os.environ.get("V3PEADD", "1") == "1"
                    j0 = 0
                    while j0 < gs:
                        j1 = min(gs, j0 + ETCH)
                        w = j1 - j0
                        psE = psX.tile([P, ETCH * HC], F32, tag="xi")
                        if _peadd:
                            nc.tensor.matmul(
                                psE[:, :w * HC], id_bf[:],
                                xj_t[:, j0:j1, :].rearrange("p a c -> p (a c)"),
                                start=True, stop=False, skip_group_check=True)
                        for j in range(j0, j1):
                            nc.tensor.matmul(
                                psE[:, (j - j0) * HC:(j - j0 + 1) * HC],
                                s_t[:, j * P:(j + 1) * P], xl_all[:, b, :],
                                start=(not _peadd), stop=((not _peadd) or j == j1 - 1),
                                skip_group_check=True)
                        if _peadd:
                            nc.scalar.activation(
                                et_t[:, j0:j1, :],
                                psE[:, :w * HC].rearrange("p (a c) -> p a c", c=HC),
                                AF.Prelu, alpha=NEG)
                        else:
                            nc.vector.tensor_tensor(
                                out=et_t[:, j0:j1, :],
                                in0=psE[:, :w * HC].rearrange("p (a c) -> p a c", c=HC),
                                in1=xj_t[:, j0:j1, :], op=OP.add)
                            nc.scalar.activation(
                                et_t[:, j0:j1, :], et_t[:, j0:j1, :],
                                AF.Prelu, alpha=NEG)
                        j0 = j1
                    # self subtile: et = prelu(xl + xr_own)
                    ssum = small.tile([P, HC], BF16, tag="ssum")
                    nc.vector.tensor_tensor(out=ssum[:], in0=xl_all[:, b, :],
                                            in1=xr_own[:, b, :], op=OP.add)
                    nc.scalar.activation(et_t[:, gs, :], ssum[:], AF.Prelu, alpha=NEG)

                    nc.vector.tensor_tensor(
                        out=et_t[:, :ns, :],
                        in0=et_t[:, :ns, :],
                        in1=attr_t[:, :ns * HC].rearrange("p (a c) -> p a c", c=HC), op=OP.mult)
                    lg_t = small.tile([P, MAXNS, HEADS], F32, tag="lg")
                    nc.vector.reduce_sum(
                        lg_t[:, :ns, :], et_t[:, :ns, :].rearrange("p j (h c) -> p j h c", h=HEADS),
                        axis=mybir.AxisListType.X)
                    v_t = work.tile([P, MAXNS, HC + HEADS], BF16, tag="vt", name="v_t")
                    nc.scalar.activation(v_t[:, :ns, HC:], lg_t[:, :ns, :], AF.Exp)
                    if gs:
                        nc.vector.tensor_tensor(
                            out=v_t[:, :gs, :HC].rearrange("p j (h c) -> p j h c", h=HEADS),
                            in0=xj_t[:, :gs, :].rearrange("p j (h c) -> p j h c", h=HEADS),
                            in1=v_t[:, :gs, HC:, None].broadcast_to([P, gs, HEADS, CH]), op=OP.mult)
                    nc.vector.tensor_tensor(
                        out=v_t[:, gs, :HC].rearrange("p (h c) -> p h c", h=HEADS),
                        in0=xr_own[:, b, :].rearrange("p (h c) -> p h c", h=HEADS),
                        in1=v_t[:, gs, HC:, None].broadcast_to([P, HEADS, CH]), op=OP.mult)

                    acc = psB.tile([P, HC + HEADS], F32, tag="acc")
                    for j in range(ns):
                        nc.tensor.matmul(acc[:], st_t[:, j, :], v_t[:, j, :],
                                         start=(j == 0), stop=(j == ns - 1))
                    nc.vector.tensor_copy(acc_all[:BN, b, :], acc[:BN, :])

                _post(45, NBLK)

                hT_prev, hT_new = hT_new, hT_prev

            # ================= out = normalize(h @ W_out + b_out) =================
            wout_t = wpool.tile([HC, EMB], F32, tag="wout")
            boutb_t = wpool.tile([P, EMB], F32, tag="boutb")
            nc.sync.dma_start(wout_t[:], wout_ext[:])
            nc.sync.dma_start(boutb_t[:], boutb_ext[:])
            for b in range(NBLK):
                cs = slice(b * BN, (b + 1) * BN)
                ps = psA.tile([P, EMB], F32, tag="mm")
                nc.tensor.matmul(ps[:BN, :], hT_prev[:HC, cs], wout_t[:], start=True, stop=True)
                osb = work.tile([P, EMB], F32, tag="osb")
                nc.vector.tensor_tensor(out=osb[:BN, :], in0=ps[:BN, :], in1=boutb_t[:BN, :], op=OP.add)
                sq_t = work.tile([P, EMB], F32, tag="osq")
                nsq_t = small.tile([P, 1], F32, tag="nsq")
                nc.scalar.activation(sq_t[:BN, :], osb[:BN, :], AF.Square, accum_out=nsq_t[:BN, :1])
                nrm_t = small.tile([P, 1], F32, tag="nrm")
                nc.scalar.activation(nrm_t[:BN, :], nsq_t[:BN, :], AF.Sqrt)
                nc.vector.tensor_scalar(out=nrm_t[:BN, :], in0=nrm_t[:BN, :], scalar1=1e-12,
                                        scalar2=None, op0=OP.max)
                recn_t = small.tile([P, 1], F32, tag="recn")
                nc.vector.reciprocal(recn_t[:BN, :], nrm_t[:BN, :])
                nc.vector.tensor_scalar(out=osb[:BN, :], in0=osb[:BN, :], scalar1=recn_t[:BN, :1],
                                        scalar2=None, op0=OP.mult)
                nc.sync.dma_start(out_ext[cs, :], osb[:BN, :])

    nc.compile()
    return nc


def _make_in_maps(inputs, idxm, dlm, dlr):
    plan = _PLAN
    x = np.asarray(inputs["x"], np.float32)
    common = {
        "win": np.asarray(inputs["W_in"], np.float32),
        "binb": _bcast(inputs["b_in"]),
        "wout": np.asarray(inputs["W_out"], np.float32),
        "boutb": _bcast(inputs["b_out"]),
    }
    for i in range(LAYERS):
        common[f"wl{i}"] = np.asarray(inputs[f"Wl{i}"], np.float32)
        common[f"wr{i}"] = np.asarray(inputs[f"Wr{i}"], np.float32)
        common[f"blb{i}"] = _bcast(inputs[f"bl{i}"])
        common[f"brb{i}"] = _bcast(inputs[f"br{i}"])
        att = np.asarray(inputs[f"att{i}"], np.float32).reshape(-1)
        common[f"attr{i}"] = np.tile(att[None, :], (P, plan.MAXNS)).astype(ml_dtypes.bfloat16)
        common[f"bob{i}"] = _bcast(inputs[f"bo{i}"])
        common[f"gb{i}"] = _bcast(inputs[f"g{i}"])
        common[f"beb{i}"] = _bcast(inputs[f"be{i}"])
    in_maps = []
    for c in range(NCORES):
        m = dict(common)
        m["xT"] = np.ascontiguousarray(x[c * NPC:(c + 1) * NPC, :].T)
        m["idxm"] = idxm[c]
        m["dlm"] = dlm[c]
        m["dlr"] = dlr[c]
        in_maps.append(m)
    return in_maps


def kernel(**inputs):
    edge_index = np.asarray(inputs["edge_index"])
    plan, _z, idxm, dlm, dlr = _preprocess(edge_index)
    nc = _build(plan)
    in_maps = _make_in_maps(inputs, idxm, dlm, dlr)
    res = run_bass_kernel_spmd(nc, in_maps, list(range(NCORES)))
    out = np.concatenate([res.results[c]["out"] for c in range(NCORES)], axis=0)
    return out.astype(np.float32)


# revision 19
# speedup vs baseline: 1.1819x; 1.1245x over previous
"""Trainium2 Bass kernel for nn_CardGNN (3-layer GATv2 message passing), v3.

Sharding: nodes partitioned across 8 NeuronCores (6250 each, 50 blocks of 125
dst nodes). Each core computes xr = h @ Wr for its OWN nodes only and the
node-major bf16 table is AllGathered (replaces v2's 8x-redundant table
compute). Per-edge source features are fetched with dma_gather (bulk 256B
rows, int16 indices, table split at row 32750), round-robining 4 SWDGE
queues. Self-loop features come from an SBUF-resident copy (no gather, and
exact per-block-slot subtile counts instead of a global max). The x_i one-hot
expansion and the x_i+x_j add both run on the TensorEngine accumulating in
PSUM; leaky-relu reads PSUM on the Scalar engine. Segment softmax +
aggregation are one-hot matmuls accumulating in PSUM.
"""
import math
import numpy as np
import ml_dtypes

import concourse.bacc as bacc
import concourse.mybir as mybir
import concourse.tile as tile
from concourse.bass_utils import run_bass_kernel_spmd

F32 = mybir.dt.float32
BF16 = mybir.dt.bfloat16
I16 = mybir.dt.int16
AF = mybir.ActivationFunctionType
OP = mybir.AluOpType

N = 50000
E = 800000
IN = 128
HID = 32
HEADS = 4
CH = 32
HC = HID * HEADS  # 128
EMB = 128
NCORES = 8
NPC = N // NCORES          # 6250 nodes per core
BN = 125                   # dst nodes per block
NBLK = NPC // BN           # 50 blocks per core
NGB = N // BN              # 400 global blocks
P = 128
SPLIT = 32750              # lo/hi table split so idx fits int16
NEG = 0.2
LAYERS = 3
import os as _os
NQ = int(_os.environ.get("V3NQ", "4"))  # SWDGE queues
ETCH = 4                   # et psum chunk, subtiles

_PLAN = None


class Plan:
    pass


def _wrap16(idx_flat):
    """dma_gather index layout: w[p, s] = idx[s*16+p], replicated to 128 rows."""
    w = idx_flat.reshape(-1, 16).T.astype(np.int16)
    return np.tile(w, (8, 1))


def _preprocess(edge_index):
    global _PLAN
    src = np.asarray(edge_index[0]).astype(np.int64)
    dst = np.asarray(edge_index[1]).astype(np.int64)
    order = np.argsort(dst, kind="stable")
    src = src[order]
    dst = dst[order]
    blk = dst // BN
    starts = np.searchsorted(blk, np.arange(NGB))
    ends = np.searchsorted(blk, np.arange(NGB) + 1)

    lo, hi = [], []
    for g in range(NGB):
        s, e = starts[g], ends[g]
        sg, dg = src[s:e], dst[s:e]
        m = sg < SPLIT
        lo.append((sg[m], dg[m]))
        hi.append((sg[~m] - SPLIT, dg[~m]))

    gl = np.zeros(NBLK, np.int64)
    gh = np.zeros(NBLK, np.int64)
    for g in range(NGB):
        b = g % NBLK
        gl[b] = max(gl[b], math.ceil(len(lo[g][0]) / P))
        gh[b] = max(gh[b], math.ceil(len(hi[g][0]) / P))
    gsub = gl + gh
    nsub = gsub + 1          # + self subtile

    plan = Plan()
    plan.gl, plan.gh, plan.gsub, plan.nsub = gl, gh, gsub, nsub
    plan.off8 = np.concatenate([[0], np.cumsum(8 * gsub)])
    plan.offn = np.concatenate([[0], np.cumsum(nsub)])
    plan.TOTIDX8 = int(plan.off8[-1])
    plan.TOTNS = int(plan.offn[-1])
    plan.MAXNS = int(nsub.max())

    idxm = np.zeros((NCORES, P, plan.TOTIDX8), np.int16)
    dlm = np.full((NCORES, P, plan.TOTNS), float(BN), ml_dtypes.bfloat16)
    dlr = np.full((NCORES, 1, plan.TOTNS * P), float(BN), ml_dtypes.bfloat16)
    selfdl = np.minimum(np.arange(P), BN).astype(np.float64)
    for g in range(NGB):
        c, b = divmod(g, NBLK)
        n0 = g * BN
        fi = np.zeros(int(gsub[b]) * P, np.int64)
        fd = np.full(int(nsub[b]) * P, float(BN), np.float64)
        (sgl, dgl), (sgh, dgh) = lo[g], hi[g]
        nl, nh = len(sgl), len(sgh)
        fi[:nl] = sgl
        fd[:nl] = (dgl - n0).astype(np.float64)
        ob = int(gl[b]) * P
        fi[ob:ob + nh] = sgh
        fd[ob:ob + nh] = (dgh - n0).astype(np.float64)
        fd[int(gsub[b]) * P:] = selfdl
        idxm[c, :, plan.off8[b]:plan.off8[b + 1]] = _wrap16(fi)
        dlm[c, :, plan.offn[b]:plan.offn[b + 1]] = \
            fd.reshape(-1, P).T.astype(ml_dtypes.bfloat16)
        dlr[c, 0, plan.offn[b] * P:plan.offn[b + 1] * P] = fd.astype(ml_dtypes.bfloat16)
    _PLAN = plan
    return plan, 0, idxm, dlm, dlr


def _bcast(v, rows=P):
    v = np.asarray(v, np.float32).reshape(-1)
    return np.tile(v[None, :], (rows, 1)).astype(np.float32)


def _build(plan, _unused=0):
    MAXNS = plan.MAXNS
    nc = bacc.Bacc(num_swdge_queues=NQ)

    # ---- I/O ----
    xT_ext = nc.declare_dram_parameter("xT", [IN, NPC], F32, isOutput=False)
    idxm_ext = nc.declare_dram_parameter("idxm", [P, plan.TOTIDX8], I16, isOutput=False)
    dlm_ext = nc.declare_dram_parameter("dlm", [P, plan.TOTNS], BF16, isOutput=False)
    dlr_ext = nc.declare_dram_parameter("dlr", [1, plan.TOTNS * P], BF16, isOutput=False)
    win_ext = nc.declare_dram_parameter("win", [IN, HID], F32, isOutput=False)
    binb_ext = nc.declare_dram_parameter("binb", [P, HID], F32, isOutput=False)
    wl_ext, wr_ext, blb_ext, brb_ext, attr_ext, bob_ext, gb_ext, beb_ext = [], [], [], [], [], [], [], []
    for i in range(LAYERS):
        ic = HID if i == 0 else HC
        wl_ext.append(nc.declare_dram_parameter(f"wl{i}", [ic, HC], BF16, isOutput=False))
        wr_ext.append(nc.declare_dram_parameter(f"wr{i}", [ic, HC], BF16, isOutput=False))
        blb_ext.append(nc.declare_dram_parameter(f"blb{i}", [P, HC], F32, isOutput=False))
        brb_ext.append(nc.declare_dram_parameter(f"brb{i}", [P, HC], F32, isOutput=False))
        attr_ext.append(nc.declare_dram_parameter(f"attr{i}", [P, MAXNS * HC], BF16, isOutput=False))
        bob_ext.append(nc.declare_dram_parameter(f"bob{i}", [P, HC], F32, isOutput=False))
        gb_ext.append(nc.declare_dram_parameter(f"gb{i}", [P, HC], F32, isOutput=False))
        beb_ext.append(nc.declare_dram_parameter(f"beb{i}", [P, HC], F32, isOutput=False))
    wout_ext = nc.declare_dram_parameter("wout", [HC, EMB], BF16, isOutput=False)
    boutb_ext = nc.declare_dram_parameter("boutb", [P, EMB], F32, isOutput=False)
    out_ext = nc.declare_dram_parameter("out", [NPC, EMB], F32, isOutput=True)

    with tile.TileContext(nc) as tc:
        with (
            tc.tile_pool(name="dram", bufs=1, space="DRAM") as dpool,
            tc.tile_pool(name="pers", bufs=1) as pers,
            tc.tile_pool(name="wpool", bufs=1) as wpool,
            tc.tile_pool(name="work", bufs=3) as work,
            tc.tile_pool(name="gbuf", bufs=4) as gbuf,
            tc.tile_pool(name="small", bufs=2) as small,
            tc.tile_pool(name="psA", bufs=2, space="PSUM") as psA,
            tc.tile_pool(name="psX", bufs=2, space="PSUM") as psX,
            tc.tile_pool(name="psB", bufs=2, space="PSUM") as psB,
        ):
            # ---- DRAM internals ----
            ag_in = [dpool.tile([NPC, HC], BF16, tag=f"ag_in{j}", name=f"ag_in{j}")
                     for j in range(LAYERS)]
            ag_out = [dpool.tile([N, HC], BF16, tag=f"ag_out{j}", name=f"ag_out{j}",
                                 addr_space="Shared") for j in range(LAYERS)]

            # ---- persistent SBUF ----
            hT_a = pers.tile([P, NPC], BF16, tag="hT_a")
            hT_b = pers.tile([P, NPC], BF16, tag="hT_b")
            xl_all = pers.tile([P, NBLK, HC], BF16, tag="xl_all")
            xr_own = pers.tile([P, NBLK, HC], BF16, tag="xr_own")
            iota_t = pers.tile([P, MAXNS, P], BF16, tag="iota")
            iotaP = pers.tile([P, MAXNS * P], BF16, tag="iotaP")
            attr_t = pers.tile([P, MAXNS * HC], BF16, tag="attr")
            id_t = pers.tile([P, P], F32, tag="ident")
            id_bf = pers.tile([P, P], BF16, tag="idbf")
            eps5_t = pers.tile([P, 1], F32, tag="eps5")
            acc_all = pers.tile([P, NBLK, HC + HEADS], F32, tag="acc_all")

            from concourse.masks import make_identity
            make_identity(nc, id_t[:])
            nc.vector.tensor_copy(id_bf[:], id_t[:])
            nc.vector.memset(eps5_t[:], 1e-5)
            nc.vector.memset(xl_all[:], 0.0)
            nc.vector.memset(xr_own[:], 0.0)
            ioi_t = pers.tile([P, MAXNS, P], mybir.dt.int32, tag="hT_b", name="ioi_t")
            nc.gpsimd.iota(ioi_t[:], pattern=[[0, MAXNS], [1, P]], base=0, channel_multiplier=0)
            nc.vector.tensor_copy(iota_t[:], ioi_t[:])
            iop_t = pers.tile([P, MAXNS * P], mybir.dt.int32, tag="hT_b", name="iop_t")
            nc.gpsimd.iota(iop_t[:], pattern=[[0, MAXNS * P]], base=0, channel_multiplier=1)
            nc.vector.tensor_copy(iotaP[:], iop_t[:])

            # ================= h0 = gelu(x @ W_in + b_in) =================
            win_t = wpool.tile([IN, HID], F32, tag="win")
            binb_t = wpool.tile([P, HID], F32, tag="binb")
            nc.sync.dma_start(win_t[:], win_ext[:])
            nc.sync.dma_start(binb_t[:], binb_ext[:])
            for b in range(NBLK):
                cs = slice(b * BN, (b + 1) * BN)
                xTb = work.tile([P, BN], F32, tag="xTb")
                nc.sync.dma_start(xTb[:IN, :], xT_ext[:, cs])
                ps = psA.tile([P, HC], F32, tag="mm")
                nc.tensor.matmul(ps[:BN, :HID], xTb[:IN, :], win_t[:], start=True, stop=True)
                h0s = work.tile([P, HID], F32, tag="h0s")
                nc.vector.tensor_tensor(out=h0s[:BN, :], in0=ps[:BN, :HID], in1=binb_t[:BN, :], op=OP.add)
                h0g = work.tile([P, HID], F32, tag="h0g")
                nc.scalar.activation(h0g[:BN, :], h0s[:BN, :], AF.Gelu)
                tp = psA.tile([HC, P], F32, tag="tp")
                nc.tensor.transpose(tp[:HID, :BN], h0g[:BN, :], id_t[:BN, :BN])
                nc.vector.tensor_copy(hT_a[:HID, cs], tp[:HID, :BN])

            hT_prev, hT_new = hT_a, hT_b

            for li in range(LAYERS):
                ic = HID if li == 0 else HC
                agi = ag_in[li]
                ago = ag_out[li]

                wl_t = wpool.tile([HC, HC], BF16, tag="wl")
                wr_t = wpool.tile([HC, HC], BF16, tag="wr")
                blb_t = wpool.tile([P, HC], F32, tag="blb")
                brb_t = wpool.tile([P, HC], F32, tag="brb")
                bob_t = wpool.tile([P, HC], F32, tag="bob")
                gb_t = wpool.tile([P, HC], F32, tag="gb")
                beb_t = wpool.tile([P, HC], F32, tag="beb")
                nc.sync.dma_start(wl_t[:ic, :], wl_ext[li][:])
                nc.sync.dma_start(wr_t[:ic, :], wr_ext[li][:])
                nc.sync.dma_start(blb_t[:], blb_ext[li][:])
                nc.sync.dma_start(brb_t[:], brb_ext[li][:])
                nc.sync.dma_start(attr_t[:], attr_ext[li][:])
                nc.sync.dma_start(bob_t[:], bob_ext[li][:])
                nc.sync.dma_start(gb_t[:], gb_ext[li][:])
                nc.sync.dma_start(beb_t[:], beb_ext[li][:])

                # ---- xr (own nodes) -> SBUF xr_own + DRAM ag_in, then AllGather ----
                for b in range(NBLK):
                    cs = slice(b * BN, (b + 1) * BN)
                    ps = psA.tile([P, HC], F32, tag="mm")
                    nc.tensor.matmul(ps[:BN, :], hT_prev[:ic, cs], wr_t[:ic, :], start=True, stop=True)
                    nc.vector.tensor_tensor(out=xr_own[:BN, b, :], in0=ps[:BN, :], in1=brb_t[:BN, :], op=OP.add)
                nc.sync.dma_start(
                    agi[:].rearrange("(b n) c -> n b c", b=NBLK), xr_own[:BN, :, :])
                nc.gpsimd.collective_compute(
                    "AllGather", OP.bypass, replica_groups=[list(range(NCORES))],
                    ins=[agi.opt()], outs=[ago.opt()],
                )

                # ---- xl (own nodes), overlaps the collective ----
                for b in range(NBLK):
                    cs = slice(b * BN, (b + 1) * BN)
                    ps = psA.tile([P, HC], F32, tag="mm")
                    nc.tensor.matmul(ps[:BN, :], hT_prev[:ic, cs], wl_t[:ic, :], start=True, stop=True)
                    nc.vector.tensor_tensor(out=xl_all[:BN, b, :], in0=ps[:BN, :], in1=blb_t[:BN, :], op=OP.add)

                # ---- layernorm + gelu + residual postlude over acc_all ----
                def _post(b0, b1, li=li, bob_t=bob_t, gb_t=gb_t, beb_t=beb_t,
                          hT_prev=hT_prev, hT_new=hT_new):
                    HB = b1 - b0
                    t_ap = acc_all[:BN, b0:b1, :HC]
                    den_t = small.tile([P, NBLK, HEADS], F32, tag="den", name="den_t")
                    nc.vector.tensor_scalar(out=den_t[:BN, :HB, :], in0=acc_all[:BN, b0:b1, HC:],
                                            scalar1=1e-16, scalar2=None, op0=OP.add)
                    rec_t = small.tile([P, NBLK, HEADS], F32, tag="rec", name="rec_t")
                    nc.vector.reciprocal(rec_t[:BN, :HB, :], den_t[:BN, :HB, :])
                    nc.vector.tensor_tensor(
                        out=t_ap.rearrange("p b (h c) -> p b h c", h=HEADS),
                        in0=t_ap.rearrange("p b (h c) -> p b h c", h=HEADS),
                        in1=rec_t[:BN, :HB, :, None].broadcast_to([BN, HB, HEADS, CH]), op=OP.mult)
                    nc.vector.tensor_tensor(
                        out=t_ap, in0=t_ap,
                        in1=bob_t[:BN, None, :].broadcast_to([BN, HB, HC]), op=OP.add)
                    mu_t = small.tile([P, NBLK], F32, tag="mu", name="mu_t")
                    nc.vector.reduce_sum(mu_t[:BN, :HB], t_ap, axis=mybir.AxisListType.X)
                    nc.vector.tensor_scalar(out=mu_t[:BN, :HB], in0=mu_t[:BN, :HB],
                                            scalar1=1.0 / HC, scalar2=None, op0=OP.mult)
                    nc.vector.tensor_tensor(
                        out=t_ap, in0=t_ap,
                        in1=mu_t[:BN, :HB, None].broadcast_to([BN, HB, HC]), op=OP.subtract)
                    var_t = small.tile([P, NBLK], F32, tag="var", name="var_t")
                    sqs_t = small.tile([P, HC], F32, tag="sqs", name="sqs_t")
                    for b in range(b0, b1):
                        nc.scalar.activation(sqs_t[:BN, :], acc_all[:BN, b, :HC], AF.Square,
                                             accum_out=var_t[:BN, b - b0:b - b0 + 1])
                    std_t = small.tile([P, NBLK], F32, tag="std", name="std_t")
                    nc.scalar.activation(std_t[:BN, :HB], var_t[:BN, :HB], AF.Sqrt,
                                         scale=1.0 / HC, bias=eps5_t[:BN, :1])
                    rstd_t = small.tile([P, NBLK], F32, tag="rstd", name="rstd_t")
                    nc.vector.reciprocal(rstd_t[:BN, :HB], std_t[:BN, :HB])
                    nc.vector.tensor_tensor(
                        out=t_ap, in0=t_ap,
                        in1=rstd_t[:BN, :HB, None].broadcast_to([BN, HB, HC]), op=OP.mult)
                    nc.vector.tensor_tensor(
                        out=t_ap, in0=t_ap,
                        in1=gb_t[:BN, None, :].broadcast_to([BN, HB, HC]), op=OP.mult)
                    nc.vector.tensor_tensor(
                        out=t_ap, in0=t_ap,
                        in1=beb_t[:BN, None, :].broadcast_to([BN, HB, HC]), op=OP.add)
                    nc.scalar.activation(t_ap, t_ap, AF.Gelu)
                    for b in range(b0, b1):
                        cs = slice(b * BN, (b + 1) * BN)
                        tp = psA.tile([HC, P], F32, tag="tp")
                        nc.tensor.transpose(tp[:, :BN], acc_all[:BN, b, :HC], id_t[:BN, :BN])
                        if li == 0:
                            nc.vector.tensor_copy(hT_new[:, cs], tp[:, :BN])
                        else:
                            nc.vector.tensor_tensor(out=hT_new[:, cs], in0=tp[:, :BN],
                                                    in1=hT_prev[:, cs], op=OP.add)

                # ---- edge blocks ----
                for b in range(NBLK):
                    if b >= 13 and (b - 13) % 10 == 0 and b < 50:
                        pc = (b - 13) // 10
                        _post(pc * 10, pc * 10 + 10)
                    if b == 48:
                        _post(40, 45)
                    ns = int(plan.nsub[b])
                    gs = int(plan.gsub[b])
                    glb = int(plan.gl[b])
                    ghb = int(plan.gh[b])
                    o8 = int(plan.off8[b])
                    on = int(plan.offn[b])

                    idxm_t = small.tile([P, 8 * MAXNS], I16, tag="idxm", bufs=3)
                    dlm_t = small.tile([P, MAXNS], BF16, tag="dlm", bufs=3)
                    dstb_t = work.tile([P, MAXNS * P], BF16, tag="dstb")
                    nc.sync.dma_start(idxm_t[:, :8 * gs], idxm_ext[:, o8:o8 + 8 * gs])
                    nc.sync.dma_start(dlm_t[:, :ns], dlm_ext[:, on:on + ns])
                    nc.sync.dma_start(dstb_t[:, :gs * P],
                                      dlr_ext[:1, on * P:(on + gs) * P].broadcast_to([P, gs * P]))

                    xj_t = gbuf.tile([P, MAXNS, HC], BF16, tag="xj")
                    if glb:
                        nc.gpsimd.dma_gather(
                            out_ap=xj_t[:, :glb, :], in_ap=ago[:SPLIT, :],
                            idxs_ap=idxm_t[:, :glb * 8],
                            num_idxs=glb * P, num_idxs_reg=glb * P, elem_size=HC,
                            single_packet=False, queue_num=(2 * b) % NQ)
                    if ghb:
                        nc.gpsimd.dma_gather(
                            out_ap=xj_t[:, glb:gs, :], in_ap=ago[SPLIT:, :],
                            idxs_ap=idxm_t[:, glb * 8:gs * 8],
                            num_idxs=ghb * P, num_idxs_reg=ghb * P, elem_size=HC,
                            single_packet=False, queue_num=(2 * b + 1) % NQ)

                    st_t = gbuf.tile([P, MAXNS, P], BF16, tag="st")
                    nc.vector.tensor_tensor(
                        out=st_t[:, :ns, :], in0=iota_t[:, :ns, :],
                        in1=dlm_t[:, :ns, None].broadcast_to([P, ns, P]), op=OP.is_equal)
                    s_t = work.tile([P, MAXNS * P], BF16, tag="s_t")
                    nc.vector.tensor_tensor(
                        out=s_t[:, :gs * P], in0=dstb_t[:, :gs * P], in1=iotaP[:, :gs * P],
                        op=OP.is_equal)

                    # x_i expansion + x_j add on PE, prelu from PSUM on Scalar
                    et_sep = work.tile([P, MAXNS, HC], BF16, tag="etsep", name="et_sep")
                    et_t = et_sep[:, :, :]
                    _peadd = _os.environ.get("V3PEADD", "1") == "1"
                    j0 = 0
                    while j0 < gs:
                        j1 = min(gs, j0 + ETCH)
                        w = j1 - j0
                        psE = psX.tile([P, ETCH * HC], F32, tag="xi")
                        if _peadd:
                            nc.tensor.matmul(
                                psE[:, :w * HC], id_bf[:],
                                xj_t[:, j0:j1, :].rearrange("p a c -> p (a c)"),
                                start=True, stop=False, skip_group_check=True)
                        for j in range(j0, j1):
                            nc.tensor.matmul(
                                psE[:, (j - j0) * HC:(j - j0 + 1) * HC],
                                s_t[:, j * P:(j + 1) * P], xl_all[:, b, :],
                                start=(not _peadd), stop=((not _peadd) or j == j1 - 1),
                                skip_group_check=True)
                        if _peadd:
                            nc.scalar.activation(
                                et_t[:, j0:j1, :],
                                psE[:, :w * HC].rearrange("p (a c) -> p a c", c=HC),
                                AF.Prelu, alpha=NEG)
                        else:
                            nc.vector.tensor_tensor(
                                out=et_t[:, j0:j1, :],
                                in0=psE[:, :w * HC].rearrange("p (a c) -> p a c", c=HC),
                                in1=xj_t[:, j0:j1, :], op=OP.add)
                            nc.scalar.activation(
                                et_t[:, j0:j1, :], et_t[:, j0:j1, :],
                                AF.Prelu, alpha=NEG)
                        j0 = j1
                    # self subtile: et = prelu(xl + xr_own)
                    ssum = small.tile([P, HC], BF16, tag="ssum")
                    nc.vector.tensor_tensor(out=ssum[:], in0=xl_all[:, b, :],
                                            in1=xr_own[:, b, :], op=OP.add)
                    nc.scalar.activation(et_t[:, gs, :], ssum[:], AF.Prelu, alpha=NEG)

                    nc.vector.tensor_tensor(
                        out=et_t[:, :ns, :],
                        in0=et_t[:, :ns, :],
                        in1=attr_t[:, :ns * HC].rearrange("p (a c) -> p a c", c=HC), op=OP.mult)
                    lg_t = small.tile([P, MAXNS, HEADS], F32, tag="lg")
                    nc.vector.reduce_sum(
                        lg_t[:, :ns, :], et_t[:, :ns, :].rearrange("p j (h c) -> p j h c", h=HEADS),
                        axis=mybir.AxisListType.X)
                    v_t = work.tile([P, MAXNS, HC + HEADS], BF16, tag="vt", name="v_t")
                    nc.scalar.activation(v_t[:, :ns, HC:], lg_t[:, :ns, :], AF.Exp)
                    if gs:
                        nc.vector.tensor_tensor(
                            out=v_t[:, :gs, :HC].rearrange("p j (h c) -> p j h c", h=HEADS),
                            in0=xj_t[:, :gs, :].rearrange("p j (h c) -> p j h c", h=HEADS),
                            in1=v_t[:, :gs, HC:, None].broadcast_to([P, gs, HEADS, CH]), op=OP.mult)
                    nc.vector.tensor_tensor(
                        out=v_t[:, gs, :HC].rearrange("p (h c) -> p h c", h=HEADS),
                        in0=xr_own[:, b, :].rearrange("p (h c) -> p h c", h=HEADS),
                        in1=v_t[:, gs, HC:, None].broadcast_to([P, HEADS, CH]), op=OP.mult)

                    acc = psB.tile([P, HC + HEADS], F32, tag="acc")
                    for j in range(ns):
                        nc.tensor.matmul(acc[:], st_t[:, j, :], v_t[:, j, :],
                                         start=(j == 0), stop=(j == ns - 1))
                    nc.vector.tensor_copy(acc_all[:BN, b, :], acc[:BN, :])

                _post(45, NBLK)

                hT_prev, hT_new = hT_new, hT_prev

            # ================= out = normalize(h @ W_out + b_out) =================
            wout_t = wpool.tile([HC, EMB], BF16, tag="wout")
            boutb_t = wpool.tile([P, EMB], F32, tag="boutb")
            nc.sync.dma_start(wout_t[:], wout_ext[:])
            nc.sync.dma_start(boutb_t[:], boutb_ext[:])
            for b in range(NBLK):
                cs = slice(b * BN, (b + 1) * BN)
                ps = psA.tile([P, EMB], F32, tag="mm")
                nc.tensor.matmul(ps[:BN, :], hT_prev[:HC, cs], wout_t[:], start=True, stop=True)
                osb = work.tile([P, EMB], F32, tag="osb")
                nc.vector.tensor_tensor(out=osb[:BN, :], in0=ps[:BN, :], in1=boutb_t[:BN, :], op=OP.add)
                sq_t = work.tile([P, EMB], F32, tag="osq")
                nsq_t = small.tile([P, 1], F32, tag="nsq")
                nc.scalar.activation(sq_t[:BN, :], osb[:BN, :], AF.Square, accum_out=nsq_t[:BN, :1])
                nrm_t = small.tile([P, 1], F32, tag="nrm")
                nc.scalar.activation(nrm_t[:BN, :], nsq_t[:BN, :], AF.Sqrt)
                nc.vector.tensor_scalar(out=nrm_t[:BN, :], in0=nrm_t[:BN, :], scalar1=1e-12,
                                        scalar2=None, op0=OP.max)
                recn_t = small.tile([P, 1], F32, tag="recn")
                nc.vector.reciprocal(recn_t[:BN, :], nrm_t[:BN, :])
                nc.vector.tensor_scalar(out=osb[:BN, :], in0=osb[:BN, :], scalar1=recn_t[:BN, :1],
                                        scalar2=None, op0=OP.mult)
                nc.sync.dma_start(out_ext[cs, :], osb[:BN, :])

    nc.compile()
    return nc


def _make_in_maps(inputs, idxm, dlm, dlr):
    plan = _PLAN
    x = np.asarray(inputs["x"], np.float32)
    common = {
        "win": np.asarray(inputs["W_in"], np.float32),
        "binb": _bcast(inputs["b_in"]),
        "wout": np.asarray(inputs["W_out"], np.float32).astype(ml_dtypes.bfloat16),
        "boutb": _bcast(inputs["b_out"]),
    }
    for i in range(LAYERS):
        common[f"wl{i}"] = np.asarray(inputs[f"Wl{i}"], np.float32).astype(ml_dtypes.bfloat16)
        common[f"wr{i}"] = np.asarray(inputs[f"Wr{i}"], np.float32).astype(ml_dtypes.bfloat16)
        common[f"blb{i}"] = _bcast(inputs[f"bl{i}"])
        common[f"brb{i}"] = _bcast(inputs[f"br{i}"])
        att = np.asarray(inputs[f"att{i}"], np.float32).reshape(-1)
        common[f"attr{i}"] = np.tile(att[None, :], (P, plan.MAXNS)).astype(ml_dtypes.bfloat16)
        common[f"bob{i}"] = _bcast(inputs[f"bo{i}"])
        common[f"gb{i}"] = _bcast(inputs[f"g{i}"])
        common[f"beb{i}"] = _bcast(inputs[f"be{i}"])
    in_maps = []
    for c in range(NCORES):
        m = dict(common)
        m["xT"] = np.ascontiguousarray(x[c * NPC:(c + 1) * NPC, :].T)
        m["idxm"] = idxm[c]
        m["dlm"] = dlm[c]
        m["dlr"] = dlr[c]
        in_maps.append(m)
    return in_maps


def kernel(**inputs):
    edge_index = np.asarray(inputs["edge_index"])
    plan, _z, idxm, dlm, dlr = _preprocess(edge_index)
    nc = _build(plan)
    in_maps = _make_in_maps(inputs, idxm, dlm, dlr)
    res = run_bass_kernel_spmd(nc, in_maps, list(range(NCORES)))
    out = np.concatenate([res.results[c]["out"] for c in range(NCORES)], axis=0)
    return out.astype(np.float32)


# revision 20
# speedup vs baseline: 1.1853x; 1.0029x over previous
"""Trainium2 Bass kernel for nn_CardGNN (3-layer GATv2 message passing), v3.

Sharding: nodes partitioned across 8 NeuronCores (6250 each, 50 blocks of 125
dst nodes). Each core computes xr = h @ Wr for its OWN nodes only and the
node-major bf16 table is AllGathered (replaces v2's 8x-redundant table
compute). Per-edge source features are fetched with dma_gather (bulk 256B
rows, int16 indices, table split at row 32750), round-robining 4 SWDGE
queues. Self-loop features come from an SBUF-resident copy (no gather, and
exact per-block-slot subtile counts instead of a global max). The x_i one-hot
expansion and the x_i+x_j add both run on the TensorEngine accumulating in
PSUM; leaky-relu reads PSUM on the Scalar engine. Segment softmax +
aggregation are one-hot matmuls accumulating in PSUM.
"""
import math
import numpy as np
import ml_dtypes

import concourse.bacc as bacc
import concourse.mybir as mybir
import concourse.tile as tile
from concourse.bass_utils import run_bass_kernel_spmd

F32 = mybir.dt.float32
BF16 = mybir.dt.bfloat16
I16 = mybir.dt.int16
AF = mybir.ActivationFunctionType
OP = mybir.AluOpType

N = 50000
E = 800000
IN = 128
HID = 32
HEADS = 4
CH = 32
HC = HID * HEADS  # 128
EMB = 128
NCORES = 8
NPC = N // NCORES          # 6250 nodes per core
BN = 125                   # dst nodes per block
NBLK = NPC // BN           # 50 blocks per core
NGB = N // BN              # 400 global blocks
P = 128
SPLIT = 32750              # lo/hi table split so idx fits int16
NEG = 0.2
LAYERS = 3
import os as _os
NQ = int(_os.environ.get("V3NQ", "4"))  # SWDGE queues
ETCH = 4                   # et psum chunk, subtiles

_PLAN = None


class Plan:
    pass


def _wrap16(idx_flat):
    """dma_gather index layout: w[p, s] = idx[s*16+p], replicated to 128 rows."""
    w = idx_flat.reshape(-1, 16).T.astype(np.int16)
    return np.tile(w, (8, 1))


def _preprocess(edge_index):
    global _PLAN
    src = np.asarray(edge_index[0]).astype(np.int64)
    dst = np.asarray(edge_index[1]).astype(np.int64)
    order = np.argsort(dst, kind="stable")
    src = src[order]
    dst = dst[order]
    blk = dst // BN
    starts = np.searchsorted(blk, np.arange(NGB))
    ends = np.searchsorted(blk, np.arange(NGB) + 1)

    lo, hi = [], []
    for g in range(NGB):
        s, e = starts[g], ends[g]
        sg, dg = src[s:e], dst[s:e]
        m = sg < SPLIT
        lo.append((sg[m], dg[m]))
        hi.append((sg[~m] - SPLIT, dg[~m]))

    gl = np.zeros(NBLK, np.int64)
    gh = np.zeros(NBLK, np.int64)
    for g in range(NGB):
        b = g % NBLK
        gl[b] = max(gl[b], math.ceil(len(lo[g][0]) / P))
        gh[b] = max(gh[b], math.ceil(len(hi[g][0]) / P))
    gsub = gl + gh
    nsub = gsub + 1          # + self subtile

    plan = Plan()
    plan.gl, plan.gh, plan.gsub, plan.nsub = gl, gh, gsub, nsub
    plan.off8 = np.concatenate([[0], np.cumsum(8 * gsub)])
    plan.offn = np.concatenate([[0], np.cumsum(nsub)])
    plan.TOTIDX8 = int(plan.off8[-1])
    plan.TOTNS = int(plan.offn[-1])
    plan.MAXNS = int(nsub.max())

    idxm = np.zeros((NCORES, P, plan.TOTIDX8), np.int16)
    dlm = np.full((NCORES, P, plan.TOTNS), float(BN), ml_dtypes.bfloat16)
    dlr = np.full((NCORES, 1, plan.TOTNS * P), float(BN), ml_dtypes.bfloat16)
    selfdl = np.minimum(np.arange(P), BN).astype(np.float64)
    for g in range(NGB):
        c, b = divmod(g, NBLK)
        n0 = g * BN
        fi = np.zeros(int(gsub[b]) * P, np.int64)
        fd = np.full(int(nsub[b]) * P, float(BN), np.float64)
        (sgl, dgl), (sgh, dgh) = lo[g], hi[g]
        nl, nh = len(sgl), len(sgh)
        fi[:nl] = sgl
        fd[:nl] = (dgl - n0).astype(np.float64)
        ob = int(gl[b]) * P
        fi[ob:ob + nh] = sgh
        fd[ob:ob + nh] = (dgh - n0).astype(np.float64)
        fd[int(gsub[b]) * P:] = selfdl
        idxm[c, :, plan.off8[b]:plan.off8[b + 1]] = _wrap16(fi)
        dlm[c, :, plan.offn[b]:plan.offn[b + 1]] = \
            fd.reshape(-1, P).T.astype(ml_dtypes.bfloat16)
        dlr[c, 0, plan.offn[b] * P:plan.offn[b + 1] * P] = fd.astype(ml_dtypes.bfloat16)
    _PLAN = plan
    return plan, 0, idxm, dlm, dlr


def _bcast(v, rows=P):
    v = np.asarray(v, np.float32).reshape(-1)
    return np.tile(v[None, :], (rows, 1)).astype(np.float32)


def _build(plan, _unused=0):
    MAXNS = plan.MAXNS
    nc = bacc.Bacc(num_swdge_queues=NQ)

    # ---- I/O ----
    xT_ext = nc.declare_dram_parameter("xT", [IN, NPC], F32, isOutput=False)
    idxm_ext = nc.declare_dram_parameter("idxm", [P, plan.TOTIDX8], I16, isOutput=False)
    dlm_ext = nc.declare_dram_parameter("dlm", [P, plan.TOTNS], BF16, isOutput=False)
    dlr_ext = nc.declare_dram_parameter("dlr", [1, plan.TOTNS * P], BF16, isOutput=False)
    win_ext = nc.declare_dram_parameter("win", [IN, HID], F32, isOutput=False)
    binb_ext = nc.declare_dram_parameter("binb", [P, HID], F32, isOutput=False)
    wl_ext, wr_ext, blb_ext, brb_ext, attr_ext, bob_ext, gb_ext, beb_ext = [], [], [], [], [], [], [], []
    for i in range(LAYERS):
        ic = HID if i == 0 else HC
        wl_ext.append(nc.declare_dram_parameter(f"wl{i}", [ic, HC], BF16, isOutput=False))
        wr_ext.append(nc.declare_dram_parameter(f"wr{i}", [ic, HC], BF16, isOutput=False))
        blb_ext.append(nc.declare_dram_parameter(f"blb{i}", [P, HC], F32, isOutput=False))
        brb_ext.append(nc.declare_dram_parameter(f"brb{i}", [P, HC], F32, isOutput=False))
        attr_ext.append(nc.declare_dram_parameter(f"attr{i}", [P, MAXNS * HC], BF16, isOutput=False))
        bob_ext.append(nc.declare_dram_parameter(f"bob{i}", [P, HC], F32, isOutput=False))
        gb_ext.append(nc.declare_dram_parameter(f"gb{i}", [P, HC], F32, isOutput=False))
        beb_ext.append(nc.declare_dram_parameter(f"beb{i}", [P, HC], F32, isOutput=False))
    wout_ext = nc.declare_dram_parameter("wout", [HC, EMB], BF16, isOutput=False)
    boutb_ext = nc.declare_dram_parameter("boutb", [P, EMB], F32, isOutput=False)
    out_ext = nc.declare_dram_parameter("out", [NPC, EMB], F32, isOutput=True)

    with tile.TileContext(nc) as tc:
        with (
            tc.tile_pool(name="dram", bufs=1, space="DRAM") as dpool,
            tc.tile_pool(name="pers", bufs=1) as pers,
            tc.tile_pool(name="wpool", bufs=1) as wpool,
            tc.tile_pool(name="work", bufs=3) as work,
            tc.tile_pool(name="gbuf", bufs=4) as gbuf,
            tc.tile_pool(name="small", bufs=2) as small,
            tc.tile_pool(name="psA", bufs=2, space="PSUM") as psA,
            tc.tile_pool(name="psX", bufs=2, space="PSUM") as psX,
            tc.tile_pool(name="psB", bufs=2, space="PSUM") as psB,
        ):
            # ---- DRAM internals ----
            ag_in = [dpool.tile([NPC, HC], BF16, tag=f"ag_in{j}", name=f"ag_in{j}")
                     for j in range(LAYERS)]
            ag_out = [dpool.tile([N, HC], BF16, tag=f"ag_out{j}", name=f"ag_out{j}",
                                 addr_space="Shared") for j in range(LAYERS)]

            # ---- persistent SBUF ----
            hT_a = pers.tile([P, NPC], BF16, tag="hT_a")
            hT_b = pers.tile([P, NPC], BF16, tag="hT_b")
            xl_all = pers.tile([P, NBLK, HC], BF16, tag="xl_all")
            xr_own = pers.tile([P, NBLK, HC], BF16, tag="xr_own")
            iota_t = pers.tile([P, MAXNS, P], BF16, tag="iota")
            iotaP = pers.tile([P, MAXNS * P], BF16, tag="iotaP")
            attr_t = pers.tile([P, MAXNS * HC], BF16, tag="attr")
            ones_bf = pers.tile([1, P], BF16, tag="ones1")
            id_t = pers.tile([P, P], F32, tag="ident")
            id_bf = pers.tile([P, P], BF16, tag="idbf")
            eps5_t = pers.tile([P, 1], F32, tag="eps5")
            acc_all = pers.tile([P, NBLK, HC + HEADS], F32, tag="acc_all")

            from concourse.masks import make_identity
            make_identity(nc, id_t[:])
            nc.vector.memset(ones_bf[:], 1.0)
            nc.vector.tensor_copy(id_bf[:], id_t[:])
            nc.vector.memset(eps5_t[:], 1e-5)
            nc.vector.memset(xl_all[:], 0.0)
            nc.vector.memset(xr_own[:], 0.0)
            ioi_t = pers.tile([P, MAXNS, P], mybir.dt.int32, tag="hT_b", name="ioi_t")
            nc.gpsimd.iota(ioi_t[:], pattern=[[0, MAXNS], [1, P]], base=0, channel_multiplier=0)
            nc.vector.tensor_copy(iota_t[:], ioi_t[:])
            iop_t = pers.tile([P, MAXNS * P], mybir.dt.int32, tag="hT_b", name="iop_t")
            nc.gpsimd.iota(iop_t[:], pattern=[[0, MAXNS * P]], base=0, channel_multiplier=1)
            nc.vector.tensor_copy(iotaP[:], iop_t[:])

            # ================= h0 = gelu(x @ W_in + b_in) =================
            win_t = wpool.tile([IN, HID], F32, tag="win")
            binb_t = wpool.tile([P, HID], F32, tag="binb")
            nc.sync.dma_start(win_t[:], win_ext[:])
            nc.sync.dma_start(binb_t[:], binb_ext[:])
            for b in range(NBLK):
                cs = slice(b * BN, (b + 1) * BN)
                xTb = work.tile([P, BN], F32, tag="xTb")
                nc.sync.dma_start(xTb[:IN, :], xT_ext[:, cs])
                ps = psA.tile([P, HC], F32, tag="mm")
                nc.tensor.matmul(ps[:BN, :HID], xTb[:IN, :], win_t[:], start=True, stop=True)
                h0s = work.tile([P, HID], F32, tag="h0s")
                nc.vector.tensor_tensor(out=h0s[:BN, :], in0=ps[:BN, :HID], in1=binb_t[:BN, :], op=OP.add)
                h0g = work.tile([P, HID], F32, tag="h0g")
                nc.scalar.activation(h0g[:BN, :], h0s[:BN, :], AF.Gelu)
                tp = psA.tile([HC, P], F32, tag="tp")
                nc.tensor.transpose(tp[:HID, :BN], h0g[:BN, :], id_t[:BN, :BN])
                nc.vector.tensor_copy(hT_a[:HID, cs], tp[:HID, :BN])

            hT_prev, hT_new = hT_a, hT_b

            for li in range(LAYERS):
                ic = HID if li == 0 else HC
                agi = ag_in[li]
                ago = ag_out[li]

                wl_t = wpool.tile([HC, HC], BF16, tag="wl")
                wr_t = wpool.tile([HC, HC], BF16, tag="wr")
                blb_t = wpool.tile([P, HC], F32, tag="blb")
                brb_t = wpool.tile([P, HC], F32, tag="brb")
                bob_t = wpool.tile([P, HC], F32, tag="bob")
                gb_t = wpool.tile([P, HC], F32, tag="gb")
                beb_t = wpool.tile([P, HC], F32, tag="beb")
                nc.sync.dma_start(wl_t[:ic, :], wl_ext[li][:])
                nc.sync.dma_start(wr_t[:ic, :], wr_ext[li][:])
                nc.sync.dma_start(blb_t[:], blb_ext[li][:])
                nc.sync.dma_start(brb_t[:], brb_ext[li][:])
                blb_bf = wpool.tile([1, HC], BF16, tag="blbbf", name="blb_bf")
                brb_bf = wpool.tile([1, HC], BF16, tag="brbbf", name="brb_bf")
                nc.vector.tensor_copy(blb_bf[:], blb_t[:1, :])
                nc.vector.tensor_copy(brb_bf[:], brb_t[:1, :])
                nc.sync.dma_start(attr_t[:], attr_ext[li][:])
                nc.sync.dma_start(bob_t[:], bob_ext[li][:])
                nc.sync.dma_start(gb_t[:], gb_ext[li][:])
                nc.sync.dma_start(beb_t[:], beb_ext[li][:])

                # ---- xr (own nodes) -> SBUF xr_own + DRAM ag_in, then AllGather ----
                for b in range(NBLK):
                    cs = slice(b * BN, (b + 1) * BN)
                    ps = psA.tile([P, HC], F32, tag="mm")
                    nc.tensor.matmul(ps[:BN, :], hT_prev[:ic, cs], wr_t[:ic, :], start=True, stop=False)
                    nc.tensor.matmul(ps[:BN, :], ones_bf[:1, :BN], brb_bf[:1, :],
                                     start=False, stop=True, skip_group_check=True)
                    nc.scalar.activation(xr_own[:BN, b, :], ps[:BN, :], AF.Copy)
                nc.sync.dma_start(
                    agi[:].rearrange("(b n) c -> n b c", b=NBLK), xr_own[:BN, :, :])
                nc.gpsimd.collective_compute(
                    "AllGather", OP.bypass, replica_groups=[list(range(NCORES))],
                    ins=[agi.opt()], outs=[ago.opt()],
                )

                # ---- xl (own nodes), overlaps the collective ----
                for b in range(NBLK):
                    cs = slice(b * BN, (b + 1) * BN)
                    ps = psA.tile([P, HC], F32, tag="mm")
                    nc.tensor.matmul(ps[:BN, :], hT_prev[:ic, cs], wl_t[:ic, :], start=True, stop=False)
                    nc.tensor.matmul(ps[:BN, :], ones_bf[:1, :BN], blb_bf[:1, :],
                                     start=False, stop=True, skip_group_check=True)
                    nc.scalar.activation(xl_all[:BN, b, :], ps[:BN, :], AF.Copy)

                # ---- layernorm + gelu + residual postlude over acc_all ----
                def _post(b0, b1, li=li, bob_t=bob_t, gb_t=gb_t, beb_t=beb_t,
                          hT_prev=hT_prev, hT_new=hT_new):
                    HB = b1 - b0
                    t_ap = acc_all[:BN, b0:b1, :HC]
                    den_t = small.tile([P, NBLK, HEADS], F32, tag="den", name="den_t")
                    nc.vector.tensor_scalar(out=den_t[:BN, :HB, :], in0=acc_all[:BN, b0:b1, HC:],
                                            scalar1=1e-16, scalar2=None, op0=OP.add)
                    rec_t = small.tile([P, NBLK, HEADS], F32, tag="rec", name="rec_t")
                    nc.vector.reciprocal(rec_t[:BN, :HB, :], den_t[:BN, :HB, :])
                    nc.vector.tensor_tensor(
                        out=t_ap.rearrange("p b (h c) -> p b h c", h=HEADS),
                        in0=t_ap.rearrange("p b (h c) -> p b h c", h=HEADS),
                        in1=rec_t[:BN, :HB, :, None].broadcast_to([BN, HB, HEADS, CH]), op=OP.mult)
                    nc.vector.tensor_tensor(
                        out=t_ap, in0=t_ap,
                        in1=bob_t[:BN, None, :].broadcast_to([BN, HB, HC]), op=OP.add)
                    mu_t = small.tile([P, NBLK], F32, tag="mu", name="mu_t")
                    nc.vector.reduce_sum(mu_t[:BN, :HB], t_ap, axis=mybir.AxisListType.X)
                    nc.vector.tensor_scalar(out=mu_t[:BN, :HB], in0=mu_t[:BN, :HB],
                                            scalar1=1.0 / HC, scalar2=None, op0=OP.mult)
                    nc.vector.tensor_tensor(
                        out=t_ap, in0=t_ap,
                        in1=mu_t[:BN, :HB, None].broadcast_to([BN, HB, HC]), op=OP.subtract)
                    var_t = small.tile([P, NBLK], F32, tag="var", name="var_t")
                    sqs_t = small.tile([P, HC], F32, tag="sqs", name="sqs_t")
                    for b in range(b0, b1):
                        nc.scalar.activation(sqs_t[:BN, :], acc_all[:BN, b, :HC], AF.Square,
                                             accum_out=var_t[:BN, b - b0:b - b0 + 1])
                    std_t = small.tile([P, NBLK], F32, tag="std", name="std_t")
                    nc.scalar.activation(std_t[:BN, :HB], var_t[:BN, :HB], AF.Sqrt,
                                         scale=1.0 / HC, bias=eps5_t[:BN, :1])
                    rstd_t = small.tile([P, NBLK], F32, tag="rstd", name="rstd_t")
                    nc.vector.reciprocal(rstd_t[:BN, :HB], std_t[:BN, :HB])
                    nc.vector.tensor_tensor(
                        out=t_ap, in0=t_ap,
                        in1=rstd_t[:BN, :HB, None].broadcast_to([BN, HB, HC]), op=OP.mult)
                    nc.vector.tensor_tensor(
                        out=t_ap, in0=t_ap,
                        in1=gb_t[:BN, None, :].broadcast_to([BN, HB, HC]), op=OP.mult)
                    nc.vector.tensor_tensor(
                        out=t_ap, in0=t_ap,
                        in1=beb_t[:BN, None, :].broadcast_to([BN, HB, HC]), op=OP.add)
                    nc.scalar.activation(t_ap, t_ap, AF.Gelu)
                    for b in range(b0, b1):
                        cs = slice(b * BN, (b + 1) * BN)
                        tp = psA.tile([HC, P], F32, tag="tp")
                        nc.tensor.transpose(tp[:, :BN], acc_all[:BN, b, :HC], id_t[:BN, :BN])
                        if li == 0:
                            nc.vector.tensor_copy(hT_new[:, cs], tp[:, :BN])
                        else:
                            nc.vector.tensor_tensor(out=hT_new[:, cs], in0=tp[:, :BN],
                                                    in1=hT_prev[:, cs], op=OP.add)

                # ---- edge blocks ----
                for b in range(NBLK):
                    if b >= 13 and (b - 13) % 10 == 0 and b < 50:
                        pc = (b - 13) // 10
                        _post(pc * 10, pc * 10 + 10)
                    if b == 48:
                        _post(40, 45)
                    ns = int(plan.nsub[b])
                    gs = int(plan.gsub[b])
                    glb = int(plan.gl[b])
                    ghb = int(plan.gh[b])
                    o8 = int(plan.off8[b])
                    on = int(plan.offn[b])

                    idxm_t = small.tile([P, 8 * MAXNS], I16, tag="idxm", bufs=3)
                    dlm_t = small.tile([P, MAXNS], BF16, tag="dlm", bufs=3)
                    dstb_t = work.tile([P, MAXNS * P], BF16, tag="dstb")
                    nc.sync.dma_start(idxm_t[:, :8 * gs], idxm_ext[:, o8:o8 + 8 * gs])
                    nc.sync.dma_start(dlm_t[:, :ns], dlm_ext[:, on:on + ns])
                    nc.sync.dma_start(dstb_t[:, :gs * P],
                                      dlr_ext[:1, on * P:(on + gs) * P].broadcast_to([P, gs * P]))

                    xj_t = gbuf.tile([P, MAXNS, HC], BF16, tag="xj")
                    if glb:
                        nc.gpsimd.dma_gather(
                            out_ap=xj_t[:, :glb, :], in_ap=ago[:SPLIT, :],
                            idxs_ap=idxm_t[:, :glb * 8],
                            num_idxs=glb * P, num_idxs_reg=glb * P, elem_size=HC,
                            single_packet=False, queue_num=(2 * b) % NQ)
                    if ghb:
                        nc.gpsimd.dma_gather(
                            out_ap=xj_t[:, glb:gs, :], in_ap=ago[SPLIT:, :],
                            idxs_ap=idxm_t[:, glb * 8:gs * 8],
                            num_idxs=ghb * P, num_idxs_reg=ghb * P, elem_size=HC,
                            single_packet=False, queue_num=(2 * b + 1) % NQ)

                    st_t = gbuf.tile([P, MAXNS, P], BF16, tag="st")
                    nc.vector.tensor_tensor(
                        out=st_t[:, :ns, :], in0=iota_t[:, :ns, :],
                        in1=dlm_t[:, :ns, None].broadcast_to([P, ns, P]), op=OP.is_equal)
                    s_t = work.tile([P, MAXNS * P], BF16, tag="s_t")
                    nc.vector.tensor_tensor(
                        out=s_t[:, :gs * P], in0=dstb_t[:, :gs * P], in1=iotaP[:, :gs * P],
                        op=OP.is_equal)

                    # x_i expansion + x_j add on PE, prelu from PSUM on Scalar
                    et_sep = work.tile([P, MAXNS, HC], BF16, tag="etsep", name="et_sep")
                    et_t = et_sep[:, :, :]
                    _peadd = _os.environ.get("V3PEADD", "1") == "1"
                    j0 = 0
                    while j0 < gs:
                        j1 = min(gs, j0 + ETCH)
                        w = j1 - j0
                        psE = psX.tile([P, ETCH * HC], F32, tag="xi")
                        if _peadd:
                            nc.tensor.matmul(
                                psE[:, :w * HC], id_bf[:],
                                xj_t[:, j0:j1, :].rearrange("p a c -> p (a c)"),
                                start=True, stop=False, skip_group_check=True)
                        for j in range(j0, j1):
                            nc.tensor.matmul(
                                psE[:, (j - j0) * HC:(j - j0 + 1) * HC],
                                s_t[:, j * P:(j + 1) * P], xl_all[:, b, :],
                                start=(not _peadd), stop=((not _peadd) or j == j1 - 1),
                                skip_group_check=True)
                        if _peadd:
                            nc.scalar.activation(
                                et_t[:, j0:j1, :],
                                psE[:, :w * HC].rearrange("p (a c) -> p a c", c=HC),
                                AF.Prelu, alpha=NEG)
                        else:
                            nc.vector.tensor_tensor(
                                out=et_t[:, j0:j1, :],
                                in0=psE[:, :w * HC].rearrange("p (a c) -> p a c", c=HC),
                                in1=xj_t[:, j0:j1, :], op=OP.add)
                            nc.scalar.activation(
                                et_t[:, j0:j1, :], et_t[:, j0:j1, :],
                                AF.Prelu, alpha=NEG)
                        j0 = j1
                    # self subtile: et = prelu(xl + xr_own)
                    ssum = small.tile([P, HC], BF16, tag="ssum")
                    nc.vector.tensor_tensor(out=ssum[:], in0=xl_all[:, b, :],
                                            in1=xr_own[:, b, :], op=OP.add)
                    nc.scalar.activation(et_t[:, gs, :], ssum[:], AF.Prelu, alpha=NEG)

                    nc.vector.tensor_tensor(
                        out=et_t[:, :ns, :],
                        in0=et_t[:, :ns, :],
                        in1=attr_t[:, :ns * HC].rearrange("p (a c) -> p a c", c=HC), op=OP.mult)
                    lg_t = small.tile([P, MAXNS, HEADS], F32, tag="lg")
                    nc.vector.reduce_sum(
                        lg_t[:, :ns, :], et_t[:, :ns, :].rearrange("p j (h c) -> p j h c", h=HEADS),
                        axis=mybir.AxisListType.X)
                    v_t = work.tile([P, MAXNS, HC + HEADS], BF16, tag="vt", name="v_t")
                    nc.scalar.activation(v_t[:, :ns, HC:], lg_t[:, :ns, :], AF.Exp)
                    if gs:
                        nc.vector.tensor_tensor(
                            out=v_t[:, :gs, :HC].rearrange("p j (h c) -> p j h c", h=HEADS),
                            in0=xj_t[:, :gs, :].rearrange("p j (h c) -> p j h c", h=HEADS),
                            in1=v_t[:, :gs, HC:, None].broadcast_to([P, gs, HEADS, CH]), op=OP.mult)
                    nc.vector.tensor_tensor(
                        out=v_t[:, gs, :HC].rearrange("p (h c) -> p h c", h=HEADS),
                        in0=xr_own[:, b, :].rearrange("p (h c) -> p h c", h=HEADS),
                        in1=v_t[:, gs, HC:, None].broadcast_to([P, HEADS, CH]), op=OP.mult)

                    acc = psB.tile([P, HC + HEADS], F32, tag="acc")
                    for j in range(ns):
                        nc.tensor.matmul(acc[:], st_t[:, j, :], v_t[:, j, :],
                                         start=(j == 0), stop=(j == ns - 1))
                    nc.scalar.activation(acc_all[:BN, b, :], acc[:BN, :], AF.Copy)

                _post(45, NBLK)

                hT_prev, hT_new = hT_new, hT_prev

            # ================= out = normalize(h @ W_out + b_out) =================
            wout_t = wpool.tile([HC, EMB], BF16, tag="wout")
            boutb_t = wpool.tile([P, EMB], F32, tag="boutb")
            nc.sync.dma_start(wout_t[:], wout_ext[:])
            nc.sync.dma_start(boutb_t[:], boutb_ext[:])
            for b in range(NBLK):
                cs = slice(b * BN, (b + 1) * BN)
                ps = psA.tile([P, EMB], F32, tag="mm")
                nc.tensor.matmul(ps[:BN, :], hT_prev[:HC, cs], wout_t[:], start=True, stop=True)
                osb = work.tile([P, EMB], F32, tag="osb")
                nc.vector.tensor_tensor(out=osb[:BN, :], in0=ps[:BN, :], in1=boutb_t[:BN, :], op=OP.add)
                sq_t = work.tile([P, EMB], F32, tag="osq")
                nsq_t = small.tile([P, 1], F32, tag="nsq")
                nc.scalar.activation(sq_t[:BN, :], osb[:BN, :], AF.Square, accum_out=nsq_t[:BN, :1])
                nrm_t = small.tile([P, 1], F32, tag="nrm")
                nc.scalar.activation(nrm_t[:BN, :], nsq_t[:BN, :], AF.Sqrt)
                nc.vector.tensor_scalar(out=nrm_t[:BN, :], in0=nrm_t[:BN, :], scalar1=1e-12,
                                        scalar2=None, op0=OP.max)
                recn_t = small.tile([P, 1], F32, tag="recn")
                nc.vector.reciprocal(recn_t[:BN, :], nrm_t[:BN, :])
                nc.vector.tensor_scalar(out=osb[:BN, :], in0=osb[:BN, :], scalar1=recn_t[:BN, :1],
                                        scalar2=None, op0=OP.mult)
                nc.sync.dma_start(out_ext[cs, :], osb[:BN, :])

    nc.compile()
    return nc


def _make_in_maps(inputs, idxm, dlm, dlr):
    plan = _PLAN
    x = np.asarray(inputs["x"], np.float32)
    common = {
        "win": np.asarray(inputs["W_in"], np.float32),
        "binb": _bcast(inputs["b_in"]),
        "wout": np.asarray(inputs["W_out"], np.float32).astype(ml_dtypes.bfloat16),
        "boutb": _bcast(inputs["b_out"]),
    }
    for i in range(LAYERS):
        common[f"wl{i}"] = np.asarray(inputs[f"Wl{i}"], np.float32).astype(ml_dtypes.bfloat16)
        common[f"wr{i}"] = np.asarray(inputs[f"Wr{i}"], np.float32).astype(ml_dtypes.bfloat16)
        common[f"blb{i}"] = _bcast(inputs[f"bl{i}"])
        common[f"brb{i}"] = _bcast(inputs[f"br{i}"])
        att = np.asarray(inputs[f"att{i}"], np.float32).reshape(-1)
        common[f"attr{i}"] = np.tile(att[None, :], (P, plan.MAXNS)).astype(ml_dtypes.bfloat16)
        common[f"bob{i}"] = _bcast(inputs[f"bo{i}"])
        common[f"gb{i}"] = _bcast(inputs[f"g{i}"])
        common[f"beb{i}"] = _bcast(inputs[f"be{i}"])
    in_maps = []
    for c in range(NCORES):
        m = dict(common)
        m["xT"] = np.ascontiguousarray(x[c * NPC:(c + 1) * NPC, :].T)
        m["idxm"] = idxm[c]
        m["dlm"] = dlm[c]
        m["dlr"] = dlr[c]
        in_maps.append(m)
    return in_maps


def kernel(**inputs):
    edge_index = np.asarray(inputs["edge_index"])
    plan, _z, idxm, dlm, dlr = _preprocess(edge_index)
    nc = _build(plan)
    in_maps = _make_in_maps(inputs, idxm, dlm, dlr)
    res = run_bass_kernel_spmd(nc, in_maps, list(range(NCORES)))
    out = np.concatenate([res.results[c]["out"] for c in range(NCORES)], axis=0)
    return out.astype(np.float32)
